# revision 15
# baseline (speedup 1.0000x reference)
"""Trainium2 Bass kernel for a 2-branch GCN siamese network (protein pairs).

Math per graph b (see reference):
    h  = leaky( A_norm @ (x @ Wg) + bg )        # GCNConv + LeakyReLU
    g  = leaky( mean_n(h) @ Wf + bf )
    xc = concat(g1, g2); 2-layer MLP + sigmoid -> scalar

Sharding: data-parallel over the batch of 8 graphs -> core b handles graph b
entirely (both branches + head) and emits a single scalar.

Device strategy:
  - A_norm is materialized dense (2048x2048, bf16) on the host, transposed,
    with the symmetric-norm coefficients folded in.  Row 2000 of A^T is all
    ones so the conv bias rides the matmul as an extra row of H; column 2047
    holds per-source row sums so Sum_t Z[j,t] falls out of the same matmuls.
  - MM1 (PE, bf16): H[n, j] = x @ Wg, accumulated in PSUM over 8 k-tiles.
  - MM2 (PE, bf16): Z^T[j, t] = sum_s H[s, j] * A^T[s, t], feature-major.
  - leaky+mean pooling fused into ACT:  leaky(z) = 0.01*z + 0.99*relu(z), so
    mean_t leaky(z) comes from Relu's accum_out plus the column-2047 sum.
  - Wf projection, head MLP and sigmoid run in fp32 on the PE (tiny).
"""

import os
import sys

import numpy as np

for _p in ("/opt/trn_rl_repo", "/root/.axon_site/_ro/trn_rl_repo"):
    if os.path.isdir(_p) and _p not in sys.path:
        sys.path.insert(0, _p)

import ml_dtypes

B, N, E, F, D = 8, 2000, 64000, 1024, 128
NT = 2048          # padded node count (targets and sources)
KT = F // 128      # 8 k-tiles over the feature dim
NB = 16            # node blocks for MM1 (15 full + one of 80)
ST = NT // 128     # 16 source tiles for MM2
TC = NT // 512     # 4 target chunks for MM2
SLOPE = 0.01

_BF16 = ml_dtypes.bfloat16

_NC = None


def _build_program():
    import concourse.bacc as bacc
    import concourse.mybir as mybir
    import concourse.tile as tile

    f32 = mybir.dt.float32
    bf16 = mybir.dt.bfloat16
    AF = mybir.ActivationFunctionType
    AL = mybir.AluOpType
    AX = mybir.AxisListType

    # Bacc (not plain Bass): its compile() runs generate_event_semaphores,
    # which splits multi-sem waits — walrus allows 1 sync wait per instruction.
    nc = bacc.Bacc()

    def ein(name, shape, dt):
        return nc.dram_tensor(name, shape, dt, kind="ExternalInput")

    xt_d = [ein("xt1", [F, N], bf16), ein("xt2", [F, N], bf16)]
    wg_d = [ein("wg1", [F, F], bf16), ein("wg2", [F, F], bf16)]
    at_d = [ein("at1", [NT, NT], bf16), ein("at2", [NT, NT], bf16)]
    bg_d = [ein("bg1", [128, KT], f32), ein("bg2", [128, KT], f32)]
    wf_d = [ein("wf1", [F, D], f32), ein("wf2", [F, D], f32)]
    bf_d = [ein("bf1", [D, 1], f32), ein("bf2", [D, 1], f32)]
    w1_d = ein("w1", [2 * D, 256], f32)
    b1_d = ein("b1", [128, 2], f32)
    w2_d = ein("w2", [256, 64], f32)
    b2_d = ein("b2", [64, 1], f32)
    wo_d = ein("wo", [64, 1], f32)
    bo_d = ein("bo", [1, 1], f32)
    out_d = nc.dram_tensor("out", [1, 1], f32, kind="ExternalOutput")

    with tile.TileContext(nc) as tc, \
            tc.tile_pool(name="p_xt", bufs=2) as p_xt, \
            tc.tile_pool(name="p_wg", bufs=2) as p_wg, \
            tc.tile_pool(name="p_h", bufs=1) as p_h, \
            tc.tile_pool(name="p_at", bufs=2) as p_at, \
            tc.tile_pool(name="p_c", bufs=1) as p_c, \
            tc.tile_pool(name="p_scr", bufs=3) as p_scr, \
            tc.tile_pool(name="p_vec", bufs=2) as p_vec, \
            tc.tile_pool(name="ps_mm1", bufs=4, space="PSUM") as ps_mm1, \
            tc.tile_pool(name="ps_mm2", bufs=2, space="PSUM") as ps_mm2, \
            tc.tile_pool(name="ps_sm", bufs=2, space="PSUM") as ps_sm:

        # ---- replicated weights (loaded once) ----
        wg_sb = []
        for br in range(2):
            w = p_wg.tile([128, KT, F], bf16, name=f"wg_sb{br}", tag="wg")
            wgr = wg_d[br][:, :].rearrange("(kt p) j -> p kt j", p=128)
            for kt in range(KT):
                nc.sync.dma_start(out=w[:, kt, :], in_=wgr[:, kt, :])
            wg_sb.append(w)

        wf_sb, bf_sb = [], []
        for br in range(2):
            wf_t = p_c.tile([128, KT, D], f32, name=f"wf_sb{br}", tag=f"wf{br}")
            nc.sync.dma_start(
                out=wf_t[:], in_=wf_d[br][:, :].rearrange("(kt p) d -> p kt d", p=128))
            wf_sb.append(wf_t)
            bf_t = p_c.tile([D, 1], f32, name=f"bf_sb{br}", tag=f"bf{br}")
            nc.sync.dma_start(out=bf_t[:], in_=bf_d[br][:, :])
            bf_sb.append(bf_t)

        w1_sb = p_c.tile([128, 2, 256], f32, name="w1_sb", tag="w1")
        nc.sync.dma_start(
            out=w1_sb[:], in_=w1_d[:, :].rearrange("(kt p) m -> p kt m", p=128))
        b1_sb = p_c.tile([128, 2], f32, name="b1_sb", tag="b1")
        nc.sync.dma_start(out=b1_sb[:], in_=b1_d[:, :])
        w2_sb = p_c.tile([128, 2, 64], f32, name="w2_sb", tag="w2")
        nc.sync.dma_start(
            out=w2_sb[:], in_=w2_d[:, :].rearrange("(kt p) m -> p kt m", p=128))
        b2_sb = p_c.tile([64, 1], f32, name="b2_sb", tag="b2")
        nc.sync.dma_start(out=b2_sb[:], in_=b2_d[:, :])
        wo_sb = p_c.tile([64, 1], f32, name="wo_sb", tag="wo")
        nc.sync.dma_start(out=wo_sb[:], in_=wo_d[:, :])
        bo_sb = p_c.tile([1, 1], f32, name="bo_sb", tag="bo")
        nc.sync.dma_start(out=bo_sb[:], in_=bo_d[:, :])

        g_vec = []
        for br in range(2):
            # ---- load x^T (sliced per node-block for fine-grained deps) ----
            xt_sb = p_xt.tile([128, KT, N], bf16, name=f"xt_sb{br}", tag="xt")
            xtr = xt_d[br][:, :].rearrange("(kt p) n -> p kt n", p=128)
            for nb in range(NB):
                n0, n1 = nb * 128, min(N, nb * 128 + 128)
                nc.sync.dma_start(out=xt_sb[:, :, n0:n1], in_=xtr[:, :, n0:n1])

            # ---- conv bias, feature-major per-partition: bgr[p, kt] and the
            #      precombined sum-term 0.01*N*bg for the pooled z-sum
            bgr_sb = p_c.tile([128, KT], f32, name=f"bgr_sb{br}", tag=f"bgr{br}")
            nc.sync.dma_start(out=bgr_sb[:], in_=bg_d[br][:, :])
            bgz_sb = p_c.tile([128, KT], f32, name=f"bgz_sb{br}", tag=f"bgz{br}")
            nc.vector.tensor_scalar_mul(bgz_sb, bgr_sb, SLOPE * float(N))

            h_sb = p_h.tile([128, ST, F], bf16, name=f"h_sb{br}", tag="h")

            # ---- MM1: H[n, j] = x @ Wg ----
            for nb in range(NB):
                n0 = nb * 128
                m = min(128, N - n0)
                pt = [ps_mm1.tile([128, 512], mybir.dt.float32,
                                  name=f"mm1ps_{br}_{nb}_{jh}", tag="mm1ps")
                      for jh in range(2)]
                for kt in range(KT):
                    for jh in range(2):
                        nc.tensor.matmul(
                            pt[jh][:m, :],
                            lhsT=xt_sb[:, kt, n0:n0 + m],
                            rhs=wg_sb[br][:, kt, jh * 512:(jh + 1) * 512],
                            start=(kt == 0), stop=(kt == KT - 1))
                for jh in range(2):
                    nc.vector.tensor_copy(
                        out=h_sb[:m, nb, jh * 512:(jh + 1) * 512], in_=pt[jh][:m, :])

            # ---- MM2: Z^T[j, t] = sum_s H[s, j] A^T[s, t]; fused pooling ----
            atr = at_d[br][:, :].rearrange("(so p) t -> p so t", p=128)
            accs = p_vec.tile([128, KT, TC + 1], f32, name=f"accs{br}", tag="accs")
            for tcx in range(TC):
                at_sb = p_at.tile([128, ST, 512], bf16,
                                  name=f"at_sb{br}_{tcx}", tag="at")
                nc.sync.dma_start(out=at_sb[:], in_=atr[:, :, tcx * 512:(tcx + 1) * 512])
                for j in range(KT):
                    zps = ps_mm2.tile([128, 512], mybir.dt.float32,
                                      name=f"mm2ps_{br}_{tcx}_{j}", tag="mm2ps")
                    for s in range(ST):
                        kp = 128 if s < ST - 1 else N - (ST - 1) * 128  # K=80 tail
                        nc.tensor.matmul(
                            zps,
                            lhsT=h_sb[:kp, s, j * 128:(j + 1) * 128],
                            rhs=at_sb[:kp, s, :],
                            start=(s == 0), stop=(s == ST - 1))
                    w = 512 if tcx < TC - 1 else N - (TC - 1) * 512  # 464 in last chunk
                    scr = p_scr.tile([128, 512], bf16,
                                     name=f"scr_{br}_{tcx}_{j}", tag="scr")
                    nc.scalar.activation(
                        out=scr[:, :w], in_=zps[:, :w], func=AF.Relu,
                        bias=bgr_sb[:, j:j + 1],
                        accum_out=accs[:, j, tcx:tcx + 1])
                    if tcx == TC - 1:
                        # column 2047 of A^T carries sum_t Z[j, t]
                        nc.vector.tensor_copy(
                            out=accs[:, j, TC:TC + 1], in_=zps[:, 511:512])

            # ---- pooled vector m[j] = 0.99*sum(relu) + 0.01*sum(z) ----
            m_sb = p_vec.tile([128, KT], f32, name=f"m_sb{br}", tag="m")
            for j in range(KT):
                s4 = p_vec.tile([128, 1], f32, name=f"s4_{br}_{j}", tag="s4")
                nc.vector.tensor_reduce(s4, accs[:, j, 0:TC], AX.X, AL.add)
                z01 = p_vec.tile([128, 1], f32, name=f"z01_{br}_{j}", tag="z01")
                nc.vector.tensor_scalar(
                    z01, accs[:, j, TC:TC + 1], SLOPE, bgz_sb[:, j:j + 1],
                    AL.mult, AL.add)
                nc.vector.tensor_scalar(
                    m_sb[:, j:j + 1], s4, 1.0 - SLOPE, z01, AL.mult, AL.add)

            # ---- g = leaky(mean @ Wf + bf)  (1/N folded into Wf host-side) ----
            gps = ps_sm.tile([128, 1], mybir.dt.float32, name=f"gps{br}", tag="sps")
            for kt in range(KT):
                nc.tensor.matmul(gps, lhsT=wf_sb[br][:, kt, :],
                                 rhs=m_sb[:, kt:kt + 1],
                                 start=(kt == 0), stop=(kt == KT - 1))
            vt = p_vec.tile([128, 1], f32, name=f"vt{br}", tag="vt")
            nc.scalar.activation(out=vt, in_=gps, func=AF.Identity, bias=bf_sb[br])
            wt = p_vec.tile([128, 1], f32, name=f"wt{br}", tag="wt")
            nc.vector.tensor_scalar_mul(wt, vt, SLOPE)
            gv = p_vec.tile([128, 1], f32, name=f"gv{br}", tag=f"gv{br}")
            nc.vector.tensor_tensor(gv, vt, wt, AL.max)
            g_vec.append(gv)

        # ---- head MLP: 256 -> 256 -> 64 -> 1, sigmoid ----
        xc1 = []
        for mb in range(2):
            xps = ps_sm.tile([128, 1], mybir.dt.float32, name=f"xps{mb}", tag="sps")
            for kt in range(2):
                nc.tensor.matmul(xps, lhsT=w1_sb[:, kt, mb * 128:(mb + 1) * 128],
                                 rhs=g_vec[kt], start=(kt == 0), stop=(kt == 1))
            vt = p_vec.tile([128, 1], f32, name=f"vh{mb}", tag="vt")
            nc.scalar.activation(out=vt, in_=xps, func=AF.Identity,
                                 bias=b1_sb[:, mb:mb + 1])
            wt = p_vec.tile([128, 1], f32, name=f"wh{mb}", tag="wt")
            nc.vector.tensor_scalar_mul(wt, vt, SLOPE)
            xv = p_vec.tile([128, 1], f32, name=f"xv{mb}", tag=f"xv{mb}")
            nc.vector.tensor_tensor(xv, vt, wt, AL.max)
            xc1.append(xv)

        x2ps = ps_sm.tile([128, 1], mybir.dt.float32, name="x2ps", tag="sps")
        for kt in range(2):
            nc.tensor.matmul(x2ps[:64], lhsT=w2_sb[:, kt, :], rhs=xc1[kt],
                             start=(kt == 0), stop=(kt == 1))
        v2 = p_vec.tile([64, 1], f32, name="v2", tag="v2")
        nc.scalar.activation(out=v2, in_=x2ps[:64], func=AF.Identity, bias=b2_sb)
        w2t = p_vec.tile([64, 1], f32, name="w2t", tag="w2t")
        nc.vector.tensor_scalar_mul(w2t, v2, SLOPE)
        xc2 = p_vec.tile([64, 1], f32, name="xc2", tag="xc2")
        nc.vector.tensor_tensor(xc2, v2, w2t, AL.max)

        ops_ = ps_sm.tile([1, 1], mybir.dt.float32, name="ops_", tag="sps")
        nc.tensor.matmul(ops_, lhsT=wo_sb, rhs=xc2, start=True, stop=True)
        osb = p_vec.tile([1, 1], f32, name="osb", tag="osb")
        nc.scalar.activation(out=osb, in_=ops_, func=AF.Sigmoid, bias=bo_sb)
        nc.sync.dma_start(out=out_d[:, :], in_=osb)

    nc.finalize()
    return nc


def _get_nc():
    global _NC
    if _NC is None:
        _NC = _build_program()
    return _NC


def _prep_branch(x, ei):
    """Host prep for one (graph, branch): x^T bf16 and the dense normalized
    adjacency, transposed, with col 2047 = row sums (gives sum_t Z)."""
    src = ei[0].astype(np.int64)
    tgt = ei[1].astype(np.int64)
    deg = (np.bincount(tgt, minlength=N) + 1).astype(np.float32)
    dinv = (1.0 / np.sqrt(deg)).astype(np.float32)
    at = np.zeros((NT, NT), np.float32)
    np.add.at(at, (src, tgt), dinv[src] * dinv[tgt])
    di = np.arange(N)
    at[di, di] += dinv * dinv
    at[:, NT - 1] = at[:, :N].sum(axis=1)    # col 2047 = row sums -> sum_t Z
    xt = np.ascontiguousarray(x.T).astype(_BF16)
    return xt, at.astype(_BF16)


def _make_in_maps(x1, ei1, x2, ei2, Wg1, bg1, Wf1, bf1, Wg2, bg2, Wf2, bf2,
                  W1, b1, W2, b2, Wo, bo):
    shared = {
        "wg1": np.ascontiguousarray(Wg1.astype(_BF16)),
        "wg2": np.ascontiguousarray(Wg2.astype(_BF16)),
        "wf1": np.ascontiguousarray((Wf1 / float(N)).astype(np.float32)),
        "wf2": np.ascontiguousarray((Wf2 / float(N)).astype(np.float32)),
        "bf1": bf1.reshape(D, 1).astype(np.float32),
        "bf2": bf2.reshape(D, 1).astype(np.float32),
        "bg1": np.ascontiguousarray(bg1.reshape(KT, 128).T.astype(np.float32)),
        "bg2": np.ascontiguousarray(bg2.reshape(KT, 128).T.astype(np.float32)),
        "w1": np.ascontiguousarray(W1.astype(np.float32)),
        "b1": np.ascontiguousarray(b1.reshape(2, 128).T.astype(np.float32)),
        "w2": np.ascontiguousarray(W2.astype(np.float32)),
        "b2": b2.reshape(64, 1).astype(np.float32),
        "wo": Wo.reshape(64, 1).astype(np.float32),
        "bo": bo.reshape(1, 1).astype(np.float32),
    }
    in_maps = []
    for b in range(B):
        m = dict(shared)
        m["xt1"], m["at1"] = _prep_branch(x1[b], ei1[b])
        m["xt2"], m["at2"] = _prep_branch(x2[b], ei2[b])
        in_maps.append(m)
    return in_maps


def kernel(**inputs):
    from concourse.bass_utils import run_bass_kernel_spmd

    nc = _get_nc()
    in_maps = _make_in_maps(**{k: np.asarray(v) for k, v in inputs.items()})
    res = run_bass_kernel_spmd(nc, in_maps, core_ids=list(range(B)))
    out = np.stack([res.results[c]["out"].reshape(1) for c in range(B)], axis=0)
    return out.astype(np.float32)


# revision 17
# speedup vs baseline: 1.0574x; 1.0574x over previous
"""Trainium2 Bass kernel for a 2-branch GCN siamese network (protein pairs).

Math per graph b (see reference):
    h  = leaky( A_norm @ (x @ Wg) + bg )        # GCNConv + LeakyReLU
    g  = leaky( mean_n(h) @ Wf + bf )
    xc = concat(g1, g2); 2-layer MLP + sigmoid -> scalar

Sharding: data-parallel over the batch of 8 graphs -> core b handles graph b
entirely (both branches + head) and emits a single scalar.

Device strategy:
  - A_norm is materialized dense (2048x2001, bf16) on the host, transposed,
    with the symmetric-norm coefficients folded in; column 2000 holds per-
    source row sums so Sum_t Z[j,t] falls out of the same matmuls.
  - MM1 (PE, bf16): H[n, j] = x @ Wg, accumulated in PSUM over 8 k-tiles.
  - MM2 (PE, bf16): Z^T[j, t] = sum_s H[s, j] * A^T[s, t], feature-major,
    K=80 partial tail tile so H's pad rows are never read.
  - leaky+mean pooling fused into ACT:  leaky(z+bg) = 0.01*(z+bg) +
    0.99*relu(z+bg); relu comes from ACT Relu with per-partition bias and
    accum_out, the linear term from the column-2000 sums.
  - Wf projection, head MLP and sigmoid run in fp32 on the PE (tiny).
"""

import os
import sys

import numpy as np

for _p in ("/opt/trn_rl_repo", "/root/.axon_site/_ro/trn_rl_repo"):
    if os.path.isdir(_p) and _p not in sys.path:
        sys.path.insert(0, _p)

import ml_dtypes

B, N, E, F, D = 8, 2000, 64000, 1024, 128
NT = 2048          # padded node count (sources)
KT = F // 128      # 8 k-tiles over the feature dim
NB = 16            # node blocks for MM1 (15 full + one of 80)
ST = NT // 128     # 16 source tiles for MM2 (last one K=80)
TC = 4             # target chunks for MM2: widths 512,512,512,465
WLAST = N - 3 * 512 + 1   # 465: 464 real targets + the col-2000 sum column
SLOPE = 0.01

_BF16 = ml_dtypes.bfloat16

_NC = None


def _build_program():
    import concourse.bacc as bacc
    import concourse.mybir as mybir
    import concourse.tile as tile

    f32 = mybir.dt.float32
    bf16 = mybir.dt.bfloat16
    AF = mybir.ActivationFunctionType
    AL = mybir.AluOpType
    AX = mybir.AxisListType

    # Bacc (not plain Bass): its compile() runs generate_event_semaphores,
    # which splits multi-sem waits — walrus allows 1 sync wait per instruction.
    nc = bacc.Bacc()

    def ein(name, shape, dt):
        return nc.dram_tensor(name, shape, dt, kind="ExternalInput")

    xt_d = [ein("xt1", [F, N], bf16), ein("xt2", [F, N], bf16)]
    wg_d = [ein("wg1", [F, F], bf16), ein("wg2", [F, F], bf16)]
    at_d = [ein("at1", [NT, N + 1], bf16), ein("at2", [NT, N + 1], bf16)]
    bg_d = [ein("bg1", [128, KT], f32), ein("bg2", [128, KT], f32)]
    wf_d = [ein("wf1", [F, D], f32), ein("wf2", [F, D], f32)]
    bf_d = [ein("bf1", [D, 1], f32), ein("bf2", [D, 1], f32)]
    w1_d = ein("w1", [2 * D, 256], f32)
    b1_d = ein("b1", [128, 2], f32)
    w2_d = ein("w2", [256, 64], f32)
    b2_d = ein("b2", [64, 1], f32)
    wo_d = ein("wo", [64, 1], f32)
    bo_d = ein("bo", [1, 1], f32)
    out_d = nc.dram_tensor("out", [1, 1], f32, kind="ExternalOutput")

    cw = [512, 512, 512, WLAST]          # MM2 chunk widths
    c0 = [0, 512, 1024, 1536]            # chunk column offsets in A^T

    with tile.TileContext(nc) as tc, \
            tc.tile_pool(name="p_xt", bufs=2) as p_xt, \
            tc.tile_pool(name="p_wg", bufs=2) as p_wg, \
            tc.tile_pool(name="p_h", bufs=1) as p_h, \
            tc.tile_pool(name="p_at", bufs=2) as p_at, \
            tc.tile_pool(name="p_c", bufs=1) as p_c, \
            tc.tile_pool(name="p_scr", bufs=3) as p_scr, \
            tc.tile_pool(name="p_vec", bufs=2) as p_vec, \
            tc.tile_pool(name="ps_mm1", bufs=4, space="PSUM") as ps_mm1, \
            tc.tile_pool(name="ps_mm2", bufs=2, space="PSUM") as ps_mm2, \
            tc.tile_pool(name="ps_sm", bufs=2, space="PSUM") as ps_sm:

        # ================= DMA issue order is the critical path =============
        # Interleave wg1 k-tiles with xt1 node-blocks so MM1(b0) starts ~2us
        # in; then bgr1 + the first two A^T chunks; bulk/late consts after.
        wg_sb = [p_wg.tile([128, KT, F], bf16, name=f"wg_sb{br}", tag="wg")
                 for br in range(2)]
        xt_sb = [p_xt.tile([128, KT, N], bf16, name=f"xt_sb{br}", tag="xt")
                 for br in range(2)]
        wgr = [wg_d[br][:, :].rearrange("(kt p) j -> p kt j", p=128)
               for br in range(2)]
        xtr = [xt_d[br][:, :].rearrange("(kt p) n -> p kt n", p=128)
               for br in range(2)]

        def load_xt(br, nb):
            n0, n1 = nb * 128, min(N, nb * 128 + 128)
            nc.sync.dma_start(out=xt_sb[br][:, :, n0:n1], in_=xtr[br][:, :, n0:n1])

        for kt in range(KT):
            nc.sync.dma_start(out=wg_sb[0][:, kt, :], in_=wgr[0][:, kt, :])
            load_xt(0, kt)
        for nb in range(KT, NB):
            load_xt(0, nb)

        bgr_sb, bgz_sb = [], []
        for br in range(2):
            bgr_t = p_c.tile([128, KT], f32, name=f"bgr_sb{br}", tag=f"bgr{br}")
            bgz_t = p_c.tile([128, KT], f32, name=f"bgz_sb{br}", tag=f"bgz{br}")
            bgr_sb.append(bgr_t)
            bgz_sb.append(bgz_t)
        nc.sync.dma_start(out=bgr_sb[0][:], in_=bg_d[0][:, :])

        # A^T chunk tiles (2 slots; pre-issue branch 0 chunks 0 and 1)
        atr = [at_d[br][:, :].rearrange("(so p) t -> p so t", p=128)
               for br in range(2)]
        at_sb = [[p_at.tile([128, ST, cw[tcx]], bf16,
                            name=f"at_sb{br}_{tcx}", tag="at")
                  for tcx in range(TC)] for br in range(2)]

        def load_at(br, tcx):
            nc.sync.dma_start(
                out=at_sb[br][tcx][:],
                in_=atr[br][:, :, c0[tcx]:c0[tcx] + cw[tcx]])

        load_at(0, 0)
        load_at(0, 1)

        # remaining (late-needed) constants
        for kt in range(KT):
            nc.sync.dma_start(out=wg_sb[1][:, kt, :], in_=wgr[1][:, kt, :])
        nc.sync.dma_start(out=bgr_sb[1][:], in_=bg_d[1][:, :])
        wf_sb, bf_sb = [], []
        for br in range(2):
            wf_t = p_c.tile([128, KT, D], f32, name=f"wf_sb{br}", tag=f"wf{br}")
            nc.sync.dma_start(
                out=wf_t[:], in_=wf_d[br][:, :].rearrange("(kt p) d -> p kt d", p=128))
            wf_sb.append(wf_t)
            bf_t = p_c.tile([D, 1], f32, name=f"bf_sb{br}", tag=f"bf{br}")
            nc.sync.dma_start(out=bf_t[:], in_=bf_d[br][:, :])
            bf_sb.append(bf_t)
        w1_sb = p_c.tile([128, 2, 256], f32, name="w1_sb", tag="w1")
        nc.sync.dma_start(
            out=w1_sb[:], in_=w1_d[:, :].rearrange("(kt p) m -> p kt m", p=128))
        b1_sb = p_c.tile([128, 2], f32, name="b1_sb", tag="b1")
        nc.sync.dma_start(out=b1_sb[:], in_=b1_d[:, :])
        w2_sb = p_c.tile([128, 2, 64], f32, name="w2_sb", tag="w2")
        nc.sync.dma_start(
            out=w2_sb[:], in_=w2_d[:, :].rearrange("(kt p) m -> p kt m", p=128))
        b2_sb = p_c.tile([64, 1], f32, name="b2_sb", tag="b2")
        nc.sync.dma_start(out=b2_sb[:], in_=b2_d[:, :])
        wo_sb = p_c.tile([64, 1], f32, name="wo_sb", tag="wo")
        nc.sync.dma_start(out=wo_sb[:], in_=wo_d[:, :])
        bo_sb = p_c.tile([1, 1], f32, name="bo_sb", tag="bo")
        nc.sync.dma_start(out=bo_sb[:], in_=bo_d[:, :])

        for br in range(2):
            nc.vector.tensor_scalar_mul(bgz_sb[br], bgr_sb[br], SLOPE * float(N))

        # ========================== compute ================================
        g_vec = []
        for br in range(2):
            if br == 1:
                for nb in range(NB):
                    load_xt(1, nb)

            h_sb = p_h.tile([128, ST, F], bf16, name=f"h_sb{br}", tag="h")

            # ---- MM1: H[n, j] = x @ Wg ----
            for nb in range(NB):
                n0 = nb * 128
                m = min(128, N - n0)
                pt = [ps_mm1.tile([128, 512], mybir.dt.float32,
                                  name=f"mm1ps_{br}_{nb}_{jh}", tag="mm1ps")
                      for jh in range(2)]
                for kt in range(KT):
                    for jh in range(2):
                        nc.tensor.matmul(
                            pt[jh][:m, :],
                            lhsT=xt_sb[br][:, kt, n0:n0 + m],
                            rhs=wg_sb[br][:, kt, jh * 512:(jh + 1) * 512],
                            start=(kt == 0), stop=(kt == KT - 1))
                for jh in range(2):
                    nc.vector.tensor_copy(
                        out=h_sb[:m, nb, jh * 512:(jh + 1) * 512], in_=pt[jh][:m, :])

            # ---- MM2: Z^T[j, t] = sum_s H[s, j] A^T[s, t]; fused pooling ----
            accs = p_vec.tile([128, KT, TC + 1], f32, name=f"accs{br}", tag="accs")
            for tcx in range(TC):
                if br == 1:
                    load_at(1, tcx)
                elif tcx >= 2:
                    load_at(0, tcx)
                at_t = at_sb[br][tcx]
                for j in range(KT):
                    zps = ps_mm2.tile([128, 512], mybir.dt.float32,
                                      name=f"mm2ps_{br}_{tcx}_{j}", tag="mm2ps")
                    for s in range(ST):
                        kp = 128 if s < ST - 1 else N - (ST - 1) * 128  # K=80 tail
                        nc.tensor.matmul(
                            zps[:, :cw[tcx]],
                            lhsT=h_sb[:kp, s, j * 128:(j + 1) * 128],
                            rhs=at_t[:kp, s, :],
                            start=(s == 0), stop=(s == ST - 1))
                    w = 512 if tcx < TC - 1 else WLAST - 1  # 464 real targets
                    scr = p_scr.tile([128, 512], bf16,
                                     name=f"scr_{br}_{tcx}_{j}", tag="scr")
                    nc.scalar.activation(
                        out=scr[:, :w], in_=zps[:, :w], func=AF.Relu,
                        bias=bgr_sb[br][:, j:j + 1],
                        accum_out=accs[:, j, tcx:tcx + 1])
                    if tcx == TC - 1:
                        # column 2000 of A^T carries sum_t Z[j, t]
                        nc.vector.tensor_copy(
                            out=accs[:, j, TC:TC + 1],
                            in_=zps[:, WLAST - 1:WLAST])

            # ---- pooled vector m[j] = 0.99*sum(relu) + 0.01*(sum(z)+N*bg) ----
            m_sb = p_vec.tile([128, KT], f32, name=f"m_sb{br}", tag="m")
            for j in range(KT):
                s4 = p_vec.tile([128, 1], f32, name=f"s4_{br}_{j}", tag="s4")
                nc.vector.tensor_reduce(s4, accs[:, j, 0:TC], AX.X, AL.add)
                z01 = p_vec.tile([128, 1], f32, name=f"z01_{br}_{j}", tag="z01")
                nc.vector.tensor_scalar(
                    z01, accs[:, j, TC:TC + 1], SLOPE, bgz_sb[br][:, j:j + 1],
                    AL.mult, AL.add)
                nc.vector.tensor_scalar(
                    m_sb[:, j:j + 1], s4, 1.0 - SLOPE, z01, AL.mult, AL.add)

            # ---- g = leaky(mean @ Wf + bf)  (1/N folded into Wf host-side) ----
            gps = ps_sm.tile([128, 1], mybir.dt.float32, name=f"gps{br}", tag="sps")
            for kt in range(KT):
                nc.tensor.matmul(gps, lhsT=wf_sb[br][:, kt, :],
                                 rhs=m_sb[:, kt:kt + 1],
                                 start=(kt == 0), stop=(kt == KT - 1))
            vt = p_vec.tile([128, 1], f32, name=f"vt{br}", tag="vt")
            nc.scalar.activation(out=vt, in_=gps, func=AF.Identity, bias=bf_sb[br])
            wt = p_vec.tile([128, 1], f32, name=f"wt{br}", tag="wt")
            nc.vector.tensor_scalar_mul(wt, vt, SLOPE)
            gv = p_vec.tile([128, 1], f32, name=f"gv{br}", tag=f"gv{br}")
            nc.vector.tensor_tensor(gv, vt, wt, AL.max)
            g_vec.append(gv)

        # ---- head MLP: 256 -> 256 -> 64 -> 1, sigmoid ----
        xc1 = []
        for mb in range(2):
            xps = ps_sm.tile([128, 1], mybir.dt.float32, name=f"xps{mb}", tag="sps")
            for kt in range(2):
                nc.tensor.matmul(xps, lhsT=w1_sb[:, kt, mb * 128:(mb + 1) * 128],
                                 rhs=g_vec[kt], start=(kt == 0), stop=(kt == 1))
            vt = p_vec.tile([128, 1], f32, name=f"vh{mb}", tag="vt")
            nc.scalar.activation(out=vt, in_=xps, func=AF.Identity,
                                 bias=b1_sb[:, mb:mb + 1])
            wt = p_vec.tile([128, 1], f32, name=f"wh{mb}", tag="wt")
            nc.vector.tensor_scalar_mul(wt, vt, SLOPE)
            xv = p_vec.tile([128, 1], f32, name=f"xv{mb}", tag=f"xv{mb}")
            nc.vector.tensor_tensor(xv, vt, wt, AL.max)
            xc1.append(xv)

        x2ps = ps_sm.tile([128, 1], mybir.dt.float32, name="x2ps", tag="sps")
        for kt in range(2):
            nc.tensor.matmul(x2ps[:64], lhsT=w2_sb[:, kt, :], rhs=xc1[kt],
                             start=(kt == 0), stop=(kt == 1))
        v2 = p_vec.tile([64, 1], f32, name="v2", tag="v2")
        nc.scalar.activation(out=v2, in_=x2ps[:64], func=AF.Identity, bias=b2_sb)
        w2t = p_vec.tile([64, 1], f32, name="w2t", tag="w2t")
        nc.vector.tensor_scalar_mul(w2t, v2, SLOPE)
        xc2 = p_vec.tile([64, 1], f32, name="xc2", tag="xc2")
        nc.vector.tensor_tensor(xc2, v2, w2t, AL.max)

        ops_ = ps_sm.tile([1, 1], mybir.dt.float32, name="ops_", tag="sps")
        nc.tensor.matmul(ops_, lhsT=wo_sb, rhs=xc2, start=True, stop=True)
        osb = p_vec.tile([1, 1], f32, name="osb", tag="osb")
        nc.scalar.activation(out=osb, in_=ops_, func=AF.Sigmoid, bias=bo_sb)
        nc.sync.dma_start(out=out_d[:, :], in_=osb)

    nc.finalize()
    return nc


def _get_nc():
    global _NC
    if _NC is None:
        _NC = _build_program()
    return _NC


def _prep_branch(x, ei):
    """Host prep for one (graph, branch): x^T bf16 and the dense normalized
    adjacency, transposed, with col 2000 = row sums (gives sum_t Z)."""
    src = ei[0].astype(np.int64)
    tgt = ei[1].astype(np.int64)
    deg = (np.bincount(tgt, minlength=N) + 1).astype(np.float32)
    dinv = (1.0 / np.sqrt(deg)).astype(np.float32)
    at = np.zeros((NT, N + 1), np.float32)
    np.add.at(at, (src, tgt), dinv[src] * dinv[tgt])
    di = np.arange(N)
    at[di, di] += dinv * dinv
    at[:, N] = at[:, :N].sum(axis=1)         # col 2000 = row sums -> sum_t Z
    xt = np.ascontiguousarray(x.T).astype(_BF16)
    return xt, at.astype(_BF16)


def _make_in_maps(x1, ei1, x2, ei2, Wg1, bg1, Wf1, bf1, Wg2, bg2, Wf2, bf2,
                  W1, b1, W2, b2, Wo, bo):
    shared = {
        "wg1": np.ascontiguousarray(Wg1.astype(_BF16)),
        "wg2": np.ascontiguousarray(Wg2.astype(_BF16)),
        "wf1": np.ascontiguousarray((Wf1 / float(N)).astype(np.float32)),
        "wf2": np.ascontiguousarray((Wf2 / float(N)).astype(np.float32)),
        "bf1": bf1.reshape(D, 1).astype(np.float32),
        "bf2": bf2.reshape(D, 1).astype(np.float32),
        "bg1": np.ascontiguousarray(bg1.reshape(KT, 128).T.astype(np.float32)),
        "bg2": np.ascontiguousarray(bg2.reshape(KT, 128).T.astype(np.float32)),
        "w1": np.ascontiguousarray(W1.astype(np.float32)),
        "b1": np.ascontiguousarray(b1.reshape(2, 128).T.astype(np.float32)),
        "w2": np.ascontiguousarray(W2.astype(np.float32)),
        "b2": b2.reshape(64, 1).astype(np.float32),
        "wo": Wo.reshape(64, 1).astype(np.float32),
        "bo": bo.reshape(1, 1).astype(np.float32),
    }
    in_maps = []
    for b in range(B):
        m = dict(shared)
        m["xt1"], m["at1"] = _prep_branch(x1[b], ei1[b])
        m["xt2"], m["at2"] = _prep_branch(x2[b], ei2[b])
        in_maps.append(m)
    return in_maps


def kernel(**inputs):
    from concourse.bass_utils import run_bass_kernel_spmd

    nc = _get_nc()
    in_maps = _make_in_maps(**{k: np.asarray(v) for k, v in inputs.items()})
    res = run_bass_kernel_spmd(nc, in_maps, core_ids=list(range(B)))
    out = np.stack([res.results[c]["out"].reshape(1) for c in range(B)], axis=0)
    return out.astype(np.float32)


# revision 18
# speedup vs baseline: 1.4925x; 1.4115x over previous
"""Trainium2 Bass kernel for a 2-branch GCN siamese network (protein pairs).

Math per graph b (see reference):
    h  = leaky( A_norm @ (x @ Wg) + bg )        # GCNConv + LeakyReLU
    g  = leaky( mean_n(h) @ Wf + bf )
    xc = concat(g1, g2); 2-layer MLP + sigmoid -> scalar

Sharding: data-parallel over the batch of 8 graphs -> core b handles graph b
entirely (both branches + head) and emits a single scalar.

Device strategy:
  - A_norm is materialized dense (2048x2001, bf16) on the host, transposed,
    with the symmetric-norm coefficients folded in; column 2000 holds per-
    source row sums so Sum_t Z[j,t] falls out of the same matmuls.
  - MM1 (PE, bf16): H[n, j] = x @ Wg, accumulated in PSUM over 8 k-tiles.
  - MM2 (PE, bf16): Z^T[j, t] = sum_s H[s, j] * A^T[s, t], feature-major,
    K=80 partial tail tile so H's pad rows are never read.
  - leaky+mean pooling fused into ACT:  leaky(z+bg) = 0.01*(z+bg) +
    0.99*relu(z+bg); relu comes from ACT Relu with per-partition bias and
    accum_out, the linear term from the column-2000 sums.
  - Wf projection, head MLP and sigmoid run in fp32 on the PE (tiny).
"""

import os
import sys

import numpy as np

for _p in ("/opt/trn_rl_repo", "/root/.axon_site/_ro/trn_rl_repo"):
    if os.path.isdir(_p) and _p not in sys.path:
        sys.path.insert(0, _p)

import ml_dtypes

B, N, E, F, D = 8, 2000, 64000, 1024, 128
NT = 2048          # padded node count (sources)
KT = F // 128      # 8 k-tiles over the feature dim
NB = 16            # node blocks for MM1 (15 full + one of 80)
ST = NT // 128     # 16 source tiles for MM2 (last one K=80)
TC = 4             # target chunks for MM2: widths 512,512,512,465
WLAST = N - 3 * 512 + 1   # 465: 464 real targets + the col-2000 sum column
SLOPE = 0.01

_BF16 = ml_dtypes.bfloat16
_FP8 = ml_dtypes.float8_e4m3

# fp8e4m3 + DoubleRow for the big A^T matmul (MM2): ~1.7x fewer PE cycles.
# A entries are in [0.016, ~3] and H ~ N(0,1) — well inside e4m3 range; the
# per-element ~4% rounding washes out to ~1e-3 after mean-pooling over 2000
# nodes. Set False to fall back to bf16 (rel err ~7e-6).
FP8_MM2 = True

_NC = None


def _build_program():
    import concourse.bacc as bacc
    import concourse.mybir as mybir
    import concourse.tile as tile

    f32 = mybir.dt.float32
    bf16 = mybir.dt.bfloat16
    AF = mybir.ActivationFunctionType
    AL = mybir.AluOpType
    AX = mybir.AxisListType

    # Bacc (not plain Bass): its compile() runs generate_event_semaphores,
    # which splits multi-sem waits — walrus allows 1 sync wait per instruction.
    nc = bacc.Bacc()

    def ein(name, shape, dt):
        return nc.dram_tensor(name, shape, dt, kind="ExternalInput")

    f8 = mybir.dt.float8e4
    hdt = f8 if FP8_MM2 else bf16
    xt_d = [ein("xt1", [F, N], bf16), ein("xt2", [F, N], bf16)]
    wg_d = [ein("wg1", [F, F], bf16), ein("wg2", [F, F], bf16)]
    at_d = [ein("at1", [NT, N + 1], hdt), ein("at2", [NT, N + 1], hdt)]
    bg_d = [ein("bg1", [128, KT], f32), ein("bg2", [128, KT], f32)]
    wf_d = [ein("wf1", [F, D], f32), ein("wf2", [F, D], f32)]
    bf_d = [ein("bf1", [D, 1], f32), ein("bf2", [D, 1], f32)]
    w1_d = ein("w1", [2 * D, 256], f32)
    b1_d = ein("b1", [128, 2], f32)
    w2_d = ein("w2", [256, 64], f32)
    b2_d = ein("b2", [64, 1], f32)
    wo_d = ein("wo", [64, 1], f32)
    bo_d = ein("bo", [1, 1], f32)
    out_d = nc.dram_tensor("out", [1, 1], f32, kind="ExternalOutput")

    cw = [512, 512, 512, WLAST]          # MM2 chunk widths
    c0 = [0, 512, 1024, 1536]            # chunk column offsets in A^T

    with tile.TileContext(nc) as tc, \
            tc.tile_pool(name="p_xt", bufs=2) as p_xt, \
            tc.tile_pool(name="p_wg", bufs=2) as p_wg, \
            tc.tile_pool(name="p_h", bufs=1) as p_h, \
            tc.tile_pool(name="p_at", bufs=2) as p_at, \
            tc.tile_pool(name="p_c", bufs=1) as p_c, \
            tc.tile_pool(name="p_scr", bufs=3) as p_scr, \
            tc.tile_pool(name="p_vec", bufs=2) as p_vec, \
            tc.tile_pool(name="ps_mm1", bufs=4, space="PSUM") as ps_mm1, \
            tc.tile_pool(name="ps_mm2", bufs=2, space="PSUM") as ps_mm2, \
            tc.tile_pool(name="ps_sm", bufs=2, space="PSUM") as ps_sm:

        # ================= DMA issue order is the critical path =============
        # Interleave wg1 k-tiles with xt1 node-blocks so MM1(b0) starts ~2us
        # in; then bgr1 + the first two A^T chunks; bulk/late consts after.
        wg_sb = [p_wg.tile([128, KT, F], bf16, name=f"wg_sb{br}", tag="wg")
                 for br in range(2)]
        xt_sb = [p_xt.tile([128, KT, N], bf16, name=f"xt_sb{br}", tag="xt")
                 for br in range(2)]
        wgr = [wg_d[br][:, :].rearrange("(kt p) j -> p kt j", p=128)
               for br in range(2)]
        xtr = [xt_d[br][:, :].rearrange("(kt p) n -> p kt n", p=128)
               for br in range(2)]

        def load_xt(br, nb):
            n0, n1 = nb * 128, min(N, nb * 128 + 128)
            nc.sync.dma_start(out=xt_sb[br][:, :, n0:n1], in_=xtr[br][:, :, n0:n1])

        for kt in range(KT):
            nc.sync.dma_start(out=wg_sb[0][:, kt, :], in_=wgr[0][:, kt, :])
            load_xt(0, kt)
        for nb in range(KT, NB):
            load_xt(0, nb)

        bgr_sb, bgz_sb = [], []
        for br in range(2):
            bgr_t = p_c.tile([128, KT], f32, name=f"bgr_sb{br}", tag=f"bgr{br}")
            bgz_t = p_c.tile([128, KT], f32, name=f"bgz_sb{br}", tag=f"bgz{br}")
            bgr_sb.append(bgr_t)
            bgz_sb.append(bgz_t)
        nc.sync.dma_start(out=bgr_sb[0][:], in_=bg_d[0][:, :])

        # A^T chunk tiles (2 slots; pre-issue branch 0 chunks 0 and 1)
        atr = [at_d[br][:, :].rearrange("(so p) t -> p so t", p=128)
               for br in range(2)]
        # fixed 512-wide tiles so the s-dim stride stays 16-aligned (DR req)
        at_sb = [[p_at.tile([128, ST, 512], hdt,
                            name=f"at_sb{br}_{tcx}", tag="at")
                  for tcx in range(TC)] for br in range(2)]

        def load_at(br, tcx):
            nc.sync.dma_start(
                out=at_sb[br][tcx][:, :, :cw[tcx]],
                in_=atr[br][:, :, c0[tcx]:c0[tcx] + cw[tcx]])

        load_at(0, 0)
        load_at(0, 1)

        # remaining (late-needed) constants
        for kt in range(KT):
            nc.sync.dma_start(out=wg_sb[1][:, kt, :], in_=wgr[1][:, kt, :])
        nc.sync.dma_start(out=bgr_sb[1][:], in_=bg_d[1][:, :])
        wf_sb, bf_sb = [], []
        for br in range(2):
            wf_t = p_c.tile([128, KT, D], f32, name=f"wf_sb{br}", tag=f"wf{br}")
            nc.sync.dma_start(
                out=wf_t[:], in_=wf_d[br][:, :].rearrange("(kt p) d -> p kt d", p=128))
            wf_sb.append(wf_t)
            bf_t = p_c.tile([D, 1], f32, name=f"bf_sb{br}", tag=f"bf{br}")
            nc.sync.dma_start(out=bf_t[:], in_=bf_d[br][:, :])
            bf_sb.append(bf_t)
        w1_sb = p_c.tile([128, 2, 256], f32, name="w1_sb", tag="w1")
        nc.sync.dma_start(
            out=w1_sb[:], in_=w1_d[:, :].rearrange("(kt p) m -> p kt m", p=128))
        b1_sb = p_c.tile([128, 2], f32, name="b1_sb", tag="b1")
        nc.sync.dma_start(out=b1_sb[:], in_=b1_d[:, :])
        w2_sb = p_c.tile([128, 2, 64], f32, name="w2_sb", tag="w2")
        nc.sync.dma_start(
            out=w2_sb[:], in_=w2_d[:, :].rearrange("(kt p) m -> p kt m", p=128))
        b2_sb = p_c.tile([64, 1], f32, name="b2_sb", tag="b2")
        nc.sync.dma_start(out=b2_sb[:], in_=b2_d[:, :])
        wo_sb = p_c.tile([64, 1], f32, name="wo_sb", tag="wo")
        nc.sync.dma_start(out=wo_sb[:], in_=wo_d[:, :])
        bo_sb = p_c.tile([1, 1], f32, name="bo_sb", tag="bo")
        nc.sync.dma_start(out=bo_sb[:], in_=bo_d[:, :])

        for br in range(2):
            nc.vector.tensor_scalar_mul(bgz_sb[br], bgr_sb[br], SLOPE * float(N))

        # ========================== compute ================================
        g_vec = []
        for br in range(2):
            if br == 1:
                for nb in range(NB):
                    load_xt(1, nb)

            h_sb = p_h.tile([128, ST, F], hdt, name=f"h_sb{br}", tag="h")
            if FP8_MM2:
                # DoubleRow pairs contract the full 16 s-tiles, so H's pad
                # rows (2000..2047) must be zero, not garbage.
                nc.vector.memset(h_sb[64:128, 15, :], 0.0)

            # ---- MM1: H[n, j] = x @ Wg ----
            for nb in range(NB):
                n0 = nb * 128
                m = min(128, N - n0)
                pt = [ps_mm1.tile([128, 512], mybir.dt.float32,
                                  name=f"mm1ps_{br}_{nb}_{jh}", tag="mm1ps")
                      for jh in range(2)]
                for kt in range(KT):
                    for jh in range(2):
                        nc.tensor.matmul(
                            pt[jh][:m, :],
                            lhsT=xt_sb[br][:, kt, n0:n0 + m],
                            rhs=wg_sb[br][:, kt, jh * 512:(jh + 1) * 512],
                            start=(kt == 0), stop=(kt == KT - 1))
                for jh in range(2):
                    nc.vector.tensor_copy(
                        out=h_sb[:m, nb, jh * 512:(jh + 1) * 512], in_=pt[jh][:m, :])

            # ---- MM2: Z^T[j, t] = sum_s H[s, j] A^T[s, t]; fused pooling ----
            accs = p_vec.tile([128, KT, TC + 1], f32, name=f"accs{br}", tag="accs")
            for tcx in range(TC):
                if br == 1:
                    load_at(1, tcx)
                elif tcx >= 2:
                    load_at(0, tcx)
                at_t = at_sb[br][tcx]
                for j in range(KT):
                    zps = ps_mm2.tile([128, 512], mybir.dt.float32,
                                      name=f"mm2ps_{br}_{tcx}_{j}", tag="mm2ps")
                    if FP8_MM2:
                        for sp in range(ST // 2):
                            nc.tensor.matmul(
                                zps[:, :cw[tcx]],
                                lhsT=h_sb[:, 2 * sp:2 * sp + 2,
                                          j * 128:(j + 1) * 128],
                                rhs=at_t[:, 2 * sp:2 * sp + 2, :cw[tcx]],
                                start=(sp == 0), stop=(sp == ST // 2 - 1),
                                perf_mode=mybir.MatmulPerfMode.DoubleRow)
                    else:
                        for s in range(ST):
                            kp = 128 if s < ST - 1 else N - (ST - 1) * 128
                            nc.tensor.matmul(
                                zps[:, :cw[tcx]],
                                lhsT=h_sb[:kp, s, j * 128:(j + 1) * 128],
                                rhs=at_t[:kp, s, :cw[tcx]],
                                start=(s == 0), stop=(s == ST - 1))
                    w = 512 if tcx < TC - 1 else WLAST - 1  # 464 real targets
                    scr = p_scr.tile([128, 512], bf16,
                                     name=f"scr_{br}_{tcx}_{j}", tag="scr")
                    nc.scalar.activation(
                        out=scr[:, :w], in_=zps[:, :w], func=AF.Relu,
                        bias=bgr_sb[br][:, j:j + 1],
                        accum_out=accs[:, j, tcx:tcx + 1])
                    if tcx == TC - 1:
                        # column 2000 of A^T carries sum_t Z[j, t]
                        nc.vector.tensor_copy(
                            out=accs[:, j, TC:TC + 1],
                            in_=zps[:, WLAST - 1:WLAST])

            # ---- pooled vector m[j] = 0.99*sum(relu) + 0.01*(sum(z)+N*bg) ----
            m_sb = p_vec.tile([128, KT], f32, name=f"m_sb{br}", tag="m")
            for j in range(KT):
                s4 = p_vec.tile([128, 1], f32, name=f"s4_{br}_{j}", tag="s4")
                nc.vector.tensor_reduce(s4, accs[:, j, 0:TC], AX.X, AL.add)
                z01 = p_vec.tile([128, 1], f32, name=f"z01_{br}_{j}", tag="z01")
                nc.vector.tensor_scalar(
                    z01, accs[:, j, TC:TC + 1], SLOPE, bgz_sb[br][:, j:j + 1],
                    AL.mult, AL.add)
                nc.vector.tensor_scalar(
                    m_sb[:, j:j + 1], s4, 1.0 - SLOPE, z01, AL.mult, AL.add)

            # ---- g = leaky(mean @ Wf + bf)  (1/N folded into Wf host-side) ----
            gps = ps_sm.tile([128, 1], mybir.dt.float32, name=f"gps{br}", tag="sps")
            for kt in range(KT):
                nc.tensor.matmul(gps, lhsT=wf_sb[br][:, kt, :],
                                 rhs=m_sb[:, kt:kt + 1],
                                 start=(kt == 0), stop=(kt == KT - 1))
            vt = p_vec.tile([128, 1], f32, name=f"vt{br}", tag="vt")
            nc.scalar.activation(out=vt, in_=gps, func=AF.Identity, bias=bf_sb[br])
            wt = p_vec.tile([128, 1], f32, name=f"wt{br}", tag="wt")
            nc.vector.tensor_scalar_mul(wt, vt, SLOPE)
            gv = p_vec.tile([128, 1], f32, name=f"gv{br}", tag=f"gv{br}")
            nc.vector.tensor_tensor(gv, vt, wt, AL.max)
            g_vec.append(gv)

        # ---- head MLP: 256 -> 256 -> 64 -> 1, sigmoid ----
        xc1 = []
        for mb in range(2):
            xps = ps_sm.tile([128, 1], mybir.dt.float32, name=f"xps{mb}", tag="sps")
            for kt in range(2):
                nc.tensor.matmul(xps, lhsT=w1_sb[:, kt, mb * 128:(mb + 1) * 128],
                                 rhs=g_vec[kt], start=(kt == 0), stop=(kt == 1))
            vt = p_vec.tile([128, 1], f32, name=f"vh{mb}", tag="vt")
            nc.scalar.activation(out=vt, in_=xps, func=AF.Identity,
                                 bias=b1_sb[:, mb:mb + 1])
            wt = p_vec.tile([128, 1], f32, name=f"wh{mb}", tag="wt")
            nc.vector.tensor_scalar_mul(wt, vt, SLOPE)
            xv = p_vec.tile([128, 1], f32, name=f"xv{mb}", tag=f"xv{mb}")
            nc.vector.tensor_tensor(xv, vt, wt, AL.max)
            xc1.append(xv)

        x2ps = ps_sm.tile([128, 1], mybir.dt.float32, name="x2ps", tag="sps")
        for kt in range(2):
            nc.tensor.matmul(x2ps[:64], lhsT=w2_sb[:, kt, :], rhs=xc1[kt],
                             start=(kt == 0), stop=(kt == 1))
        v2 = p_vec.tile([64, 1], f32, name="v2", tag="v2")
        nc.scalar.activation(out=v2, in_=x2ps[:64], func=AF.Identity, bias=b2_sb)
        w2t = p_vec.tile([64, 1], f32, name="w2t", tag="w2t")
        nc.vector.tensor_scalar_mul(w2t, v2, SLOPE)
        xc2 = p_vec.tile([64, 1], f32, name="xc2", tag="xc2")
        nc.vector.tensor_tensor(xc2, v2, w2t, AL.max)

        ops_ = ps_sm.tile([1, 1], mybir.dt.float32, name="ops_", tag="sps")
        nc.tensor.matmul(ops_, lhsT=wo_sb, rhs=xc2, start=True, stop=True)
        osb = p_vec.tile([1, 1], f32, name="osb", tag="osb")
        nc.scalar.activation(out=osb, in_=ops_, func=AF.Sigmoid, bias=bo_sb)
        nc.sync.dma_start(out=out_d[:, :], in_=osb)

    nc.finalize()
    return nc


def _get_nc():
    global _NC
    if _NC is None:
        _NC = _build_program()
    return _NC


def _prep_branch(x, ei):
    """Host prep for one (graph, branch): x^T bf16 and the dense normalized
    adjacency, transposed, with col 2000 = row sums (gives sum_t Z)."""
    src = ei[0].astype(np.int64)
    tgt = ei[1].astype(np.int64)
    deg = (np.bincount(tgt, minlength=N) + 1).astype(np.float32)
    dinv = (1.0 / np.sqrt(deg)).astype(np.float32)
    at = np.zeros((NT, N + 1), np.float32)
    np.add.at(at, (src, tgt), dinv[src] * dinv[tgt])
    di = np.arange(N)
    at[di, di] += dinv * dinv
    at[:, N] = at[:, :N].sum(axis=1)         # col 2000 = row sums -> sum_t Z
    xt = np.ascontiguousarray(x.T).astype(_BF16)
    return xt, at.astype(_FP8 if FP8_MM2 else _BF16)


def _make_in_maps(x1, ei1, x2, ei2, Wg1, bg1, Wf1, bf1, Wg2, bg2, Wf2, bf2,
                  W1, b1, W2, b2, Wo, bo):
    shared = {
        "wg1": np.ascontiguousarray(Wg1.astype(_BF16)),
        "wg2": np.ascontiguousarray(Wg2.astype(_BF16)),
        "wf1": np.ascontiguousarray((Wf1 / float(N)).astype(np.float32)),
        "wf2": np.ascontiguousarray((Wf2 / float(N)).astype(np.float32)),
        "bf1": bf1.reshape(D, 1).astype(np.float32),
        "bf2": bf2.reshape(D, 1).astype(np.float32),
        "bg1": np.ascontiguousarray(bg1.reshape(KT, 128).T.astype(np.float32)),
        "bg2": np.ascontiguousarray(bg2.reshape(KT, 128).T.astype(np.float32)),
        "w1": np.ascontiguousarray(W1.astype(np.float32)),
        "b1": np.ascontiguousarray(b1.reshape(2, 128).T.astype(np.float32)),
        "w2": np.ascontiguousarray(W2.astype(np.float32)),
        "b2": b2.reshape(64, 1).astype(np.float32),
        "wo": Wo.reshape(64, 1).astype(np.float32),
        "bo": bo.reshape(1, 1).astype(np.float32),
    }
    in_maps = []
    for b in range(B):
        m = dict(shared)
        m["xt1"], m["at1"] = _prep_branch(x1[b], ei1[b])
        m["xt2"], m["at2"] = _prep_branch(x2[b], ei2[b])
        in_maps.append(m)
    return in_maps


def kernel(**inputs):
    from concourse.bass_utils import run_bass_kernel_spmd

    nc = _get_nc()
    in_maps = _make_in_maps(**{k: np.asarray(v) for k, v in inputs.items()})
    res = run_bass_kernel_spmd(nc, in_maps, core_ids=list(range(B)))
    out = np.stack([res.results[c]["out"].reshape(1) for c in range(B)], axis=0)
    return out.astype(np.float32)


# revision 19
# speedup vs baseline: 1.8572x; 1.2443x over previous
"""Trainium2 Bass kernel for a 2-branch GCN siamese network (protein pairs).

Math per graph b (see reference):
    h  = leaky( A_norm @ (x @ Wg) + bg )        # GCNConv + LeakyReLU
    g  = leaky( mean_n(h) @ Wf + bf )
    xc = concat(g1, g2); 2-layer MLP + sigmoid -> scalar

Sharding: data-parallel over the batch of 8 graphs -> core b handles graph b
entirely (both branches + head) and emits a single scalar.

Device strategy:
  - A_norm is materialized dense (2048x2001, bf16) on the host, transposed,
    with the symmetric-norm coefficients folded in; column 2000 holds per-
    source row sums so Sum_t Z[j,t] falls out of the same matmuls.
  - MM1 (PE, bf16): H[n, j] = x @ Wg, accumulated in PSUM over 8 k-tiles.
  - MM2 (PE, bf16): Z^T[j, t] = sum_s H[s, j] * A^T[s, t], feature-major,
    K=80 partial tail tile so H's pad rows are never read.
  - leaky+mean pooling fused into ACT:  leaky(z+bg) = 0.01*(z+bg) +
    0.99*relu(z+bg); relu comes from ACT Relu with per-partition bias and
    accum_out, the linear term from the column-2000 sums.
  - Wf projection, head MLP and sigmoid run in fp32 on the PE (tiny).
"""

import os
import sys

import numpy as np

for _p in ("/opt/trn_rl_repo", "/root/.axon_site/_ro/trn_rl_repo"):
    if os.path.isdir(_p) and _p not in sys.path:
        sys.path.insert(0, _p)

import ml_dtypes

B, N, E, F, D = 8, 2000, 64000, 1024, 128
NT = 2048          # padded node count (sources)
KT = F // 128      # 8 k-tiles over the feature dim
NB = 16            # node blocks for MM1 (15 full + one of 80)
ST = NT // 128     # 16 source tiles for MM2 (last one K=80)
TC = 4             # target chunks for MM2: widths 512,512,512,465
WLAST = N - 3 * 512 + 1   # 465: 464 real targets + the col-2000 sum column
SLOPE = 0.01

_BF16 = ml_dtypes.bfloat16
_FP8 = ml_dtypes.float8_e4m3

# fp8e4m3 + DoubleRow for the big A^T matmul (MM2): ~1.7x fewer PE cycles.
# A entries are in [0.016, ~3] and H ~ N(0,1) — well inside e4m3 range; the
# per-element ~4% rounding washes out to ~1e-3 after mean-pooling over 2000
# nodes. Set False to fall back to bf16 (rel err ~7e-6).
FP8_MM2 = True
# fp8 + DoubleRow for x@Wg as well (numpy model: max batch rel err 1.8e-4).
FP8_MM1 = True

_NC = None


def _build_program():
    import concourse.bacc as bacc
    import concourse.mybir as mybir
    import concourse.tile as tile

    f32 = mybir.dt.float32
    bf16 = mybir.dt.bfloat16
    AF = mybir.ActivationFunctionType
    AL = mybir.AluOpType
    AX = mybir.AxisListType

    # Bacc (not plain Bass): its compile() runs generate_event_semaphores,
    # which splits multi-sem waits — walrus allows 1 sync wait per instruction.
    nc = bacc.Bacc()

    def ein(name, shape, dt):
        return nc.dram_tensor(name, shape, dt, kind="ExternalInput")

    f8 = mybir.dt.float8e4
    hdt = f8 if FP8_MM2 else bf16
    xdt = f8 if FP8_MM1 else bf16
    xt_d = [ein("xt1", [F, N], xdt), ein("xt2", [F, N], xdt)]
    wg_d = [ein("wg1", [F, F], xdt), ein("wg2", [F, F], xdt)]
    at_d = [ein("at1", [NT, N + 1], hdt), ein("at2", [NT, N + 1], hdt)]
    bg_d = [ein("bg1", [128, KT], f32), ein("bg2", [128, KT], f32)]
    wf_d = [ein("wf1", [F, D], f32), ein("wf2", [F, D], f32)]
    bf_d = [ein("bf1", [D, 1], f32), ein("bf2", [D, 1], f32)]
    w1_d = ein("w1", [2 * D, 256], f32)
    b1_d = ein("b1", [128, 2], f32)
    w2_d = ein("w2", [256, 64], f32)
    b2_d = ein("b2", [64, 1], f32)
    wo_d = ein("wo", [64, 1], f32)
    bo_d = ein("bo", [1, 1], f32)
    out_d = nc.dram_tensor("out", [1, 1], f32, kind="ExternalOutput")

    cw = [512, 512, 512, WLAST]          # MM2 chunk widths
    c0 = [0, 512, 1024, 1536]            # chunk column offsets in A^T

    with tile.TileContext(nc) as tc, \
            tc.tile_pool(name="p_xt", bufs=2) as p_xt, \
            tc.tile_pool(name="p_wg", bufs=2) as p_wg, \
            tc.tile_pool(name="p_h", bufs=1) as p_h, \
            tc.tile_pool(name="p_at", bufs=2) as p_at, \
            tc.tile_pool(name="p_c", bufs=1) as p_c, \
            tc.tile_pool(name="p_scr", bufs=3) as p_scr, \
            tc.tile_pool(name="p_vec", bufs=2) as p_vec, \
            tc.tile_pool(name="ps_mm1", bufs=4, space="PSUM") as ps_mm1, \
            tc.tile_pool(name="ps_mm2", bufs=2, space="PSUM") as ps_mm2, \
            tc.tile_pool(name="ps_sm", bufs=2, space="PSUM") as ps_sm:

        # ================= DMA issue order is the critical path =============
        # Interleave wg1 k-tiles with xt1 node-blocks so MM1(b0) starts ~2us
        # in; then bgr1 + the first two A^T chunks; bulk/late consts after.
        wg_sb = [p_wg.tile([128, KT, F], xdt, name=f"wg_sb{br}", tag="wg")
                 for br in range(2)]
        xt_sb = [p_xt.tile([128, KT, N], xdt, name=f"xt_sb{br}", tag="xt")
                 for br in range(2)]
        wgr = [wg_d[br][:, :].rearrange("(kt p) j -> p kt j", p=128)
               for br in range(2)]
        xtr = [xt_d[br][:, :].rearrange("(kt p) n -> p kt n", p=128)
               for br in range(2)]

        def load_xt(br, nb):
            n0, n1 = nb * 128, min(N, nb * 128 + 128)
            nc.sync.dma_start(out=xt_sb[br][:, :, n0:n1], in_=xtr[br][:, :, n0:n1])

        for kt in range(KT):
            nc.sync.dma_start(out=wg_sb[0][:, kt, :], in_=wgr[0][:, kt, :])
            load_xt(0, kt)
        for nb in range(KT, NB):
            load_xt(0, nb)

        bgr_sb, bgz_sb = [], []
        for br in range(2):
            bgr_t = p_c.tile([128, KT], f32, name=f"bgr_sb{br}", tag=f"bgr{br}")
            bgz_t = p_c.tile([128, KT], f32, name=f"bgz_sb{br}", tag=f"bgz{br}")
            bgr_sb.append(bgr_t)
            bgz_sb.append(bgz_t)
        nc.sync.dma_start(out=bgr_sb[0][:], in_=bg_d[0][:, :])

        # A^T chunk tiles (2 slots; pre-issue branch 0 chunks 0 and 1)
        atr = [at_d[br][:, :].rearrange("(so p) t -> p so t", p=128)
               for br in range(2)]
        # fixed 512-wide tiles so the s-dim stride stays 16-aligned (DR req)
        at_sb = [[p_at.tile([128, ST, 512], hdt,
                            name=f"at_sb{br}_{tcx}", tag="at")
                  for tcx in range(TC)] for br in range(2)]

        def load_at(br, tcx):
            nc.sync.dma_start(
                out=at_sb[br][tcx][:, :, :cw[tcx]],
                in_=atr[br][:, :, c0[tcx]:c0[tcx] + cw[tcx]])

        load_at(0, 0)
        load_at(0, 1)

        # remaining (late-needed) constants
        for kt in range(KT):
            nc.sync.dma_start(out=wg_sb[1][:, kt, :], in_=wgr[1][:, kt, :])
        nc.sync.dma_start(out=bgr_sb[1][:], in_=bg_d[1][:, :])
        wf_sb, bf_sb = [], []
        for br in range(2):
            wf_t = p_c.tile([128, KT, D], f32, name=f"wf_sb{br}", tag=f"wf{br}")
            nc.sync.dma_start(
                out=wf_t[:], in_=wf_d[br][:, :].rearrange("(kt p) d -> p kt d", p=128))
            wf_sb.append(wf_t)
            bf_t = p_c.tile([D, 1], f32, name=f"bf_sb{br}", tag=f"bf{br}")
            nc.sync.dma_start(out=bf_t[:], in_=bf_d[br][:, :])
            bf_sb.append(bf_t)
        w1_sb = p_c.tile([128, 2, 256], f32, name="w1_sb", tag="w1")
        nc.sync.dma_start(
            out=w1_sb[:], in_=w1_d[:, :].rearrange("(kt p) m -> p kt m", p=128))
        b1_sb = p_c.tile([128, 2], f32, name="b1_sb", tag="b1")
        nc.sync.dma_start(out=b1_sb[:], in_=b1_d[:, :])
        w2_sb = p_c.tile([128, 2, 64], f32, name="w2_sb", tag="w2")
        nc.sync.dma_start(
            out=w2_sb[:], in_=w2_d[:, :].rearrange("(kt p) m -> p kt m", p=128))
        b2_sb = p_c.tile([64, 1], f32, name="b2_sb", tag="b2")
        nc.sync.dma_start(out=b2_sb[:], in_=b2_d[:, :])
        wo_sb = p_c.tile([64, 1], f32, name="wo_sb", tag="wo")
        nc.sync.dma_start(out=wo_sb[:], in_=wo_d[:, :])
        bo_sb = p_c.tile([1, 1], f32, name="bo_sb", tag="bo")
        nc.sync.dma_start(out=bo_sb[:], in_=bo_d[:, :])

        for br in range(2):
            nc.vector.tensor_scalar_mul(bgz_sb[br], bgr_sb[br], SLOPE * float(N))

        # ========================== compute ================================
        g_vec = []
        for br in range(2):
            if br == 1:
                for nb in range(NB):
                    load_xt(1, nb)

            h_sb = p_h.tile([128, ST, F], hdt, name=f"h_sb{br}", tag="h")
            if FP8_MM2:
                # DoubleRow pairs contract the full 16 s-tiles, so H's pad
                # rows (2000..2047) must be zero, not garbage.
                nc.vector.memset(h_sb[64:128, 15, :], 0.0)

            # ---- MM1: H[n, j] = x @ Wg ----
            for nb in range(NB):
                n0 = nb * 128
                m = min(128, N - n0)
                pt = [ps_mm1.tile([128, 512], mybir.dt.float32,
                                  name=f"mm1ps_{br}_{nb}_{jh}", tag="mm1ps")
                      for jh in range(2)]
                if FP8_MM1:
                    for kp in range(KT // 2):
                        for jh in range(2):
                            nc.tensor.matmul(
                                pt[jh][:m, :],
                                lhsT=xt_sb[br][:, 2 * kp:2 * kp + 2, n0:n0 + m],
                                rhs=wg_sb[br][:, 2 * kp:2 * kp + 2,
                                              jh * 512:(jh + 1) * 512],
                                start=(kp == 0), stop=(kp == KT // 2 - 1),
                                perf_mode=mybir.MatmulPerfMode.DoubleRow)
                else:
                    for kt in range(KT):
                        for jh in range(2):
                            nc.tensor.matmul(
                                pt[jh][:m, :],
                                lhsT=xt_sb[br][:, kt, n0:n0 + m],
                                rhs=wg_sb[br][:, kt, jh * 512:(jh + 1) * 512],
                                start=(kt == 0), stop=(kt == KT - 1))
                for jh in range(2):
                    nc.vector.tensor_copy(
                        out=h_sb[:m, nb, jh * 512:(jh + 1) * 512], in_=pt[jh][:m, :])

            # ---- MM2: Z^T[j, t] = sum_s H[s, j] A^T[s, t]; fused pooling ----
            accs = p_vec.tile([128, KT, TC + 1], f32, name=f"accs{br}", tag="accs")
            for tcx in range(TC):
                if br == 1:
                    load_at(1, tcx)
                elif tcx >= 2:
                    load_at(0, tcx)
                at_t = at_sb[br][tcx]
                for j in range(KT):
                    zps = ps_mm2.tile([128, 512], mybir.dt.float32,
                                      name=f"mm2ps_{br}_{tcx}_{j}", tag="mm2ps")
                    if FP8_MM2:
                        for sp in range(ST // 2):
                            nc.tensor.matmul(
                                zps[:, :cw[tcx]],
                                lhsT=h_sb[:, 2 * sp:2 * sp + 2,
                                          j * 128:(j + 1) * 128],
                                rhs=at_t[:, 2 * sp:2 * sp + 2, :cw[tcx]],
                                start=(sp == 0), stop=(sp == ST // 2 - 1),
                                perf_mode=mybir.MatmulPerfMode.DoubleRow)
                    else:
                        for s in range(ST):
                            kp = 128 if s < ST - 1 else N - (ST - 1) * 128
                            nc.tensor.matmul(
                                zps[:, :cw[tcx]],
                                lhsT=h_sb[:kp, s, j * 128:(j + 1) * 128],
                                rhs=at_t[:kp, s, :cw[tcx]],
                                start=(s == 0), stop=(s == ST - 1))
                    w = 512 if tcx < TC - 1 else WLAST - 1  # 464 real targets
                    scr = p_scr.tile([128, 512], bf16,
                                     name=f"scr_{br}_{tcx}_{j}", tag="scr")
                    nc.scalar.activation(
                        out=scr[:, :w], in_=zps[:, :w], func=AF.Relu,
                        bias=bgr_sb[br][:, j:j + 1],
                        accum_out=accs[:, j, tcx:tcx + 1])
                    if tcx == TC - 1:
                        # column 2000 of A^T carries sum_t Z[j, t]
                        nc.vector.tensor_copy(
                            out=accs[:, j, TC:TC + 1],
                            in_=zps[:, WLAST - 1:WLAST])

            # ---- pooled vector m[j] = 0.99*sum(relu) + 0.01*(sum(z)+N*bg) ----
            m_sb = p_vec.tile([128, KT], f32, name=f"m_sb{br}", tag="m")
            for j in range(KT):
                s4 = p_vec.tile([128, 1], f32, name=f"s4_{br}_{j}", tag="s4")
                nc.vector.tensor_reduce(s4, accs[:, j, 0:TC], AX.X, AL.add)
                z01 = p_vec.tile([128, 1], f32, name=f"z01_{br}_{j}", tag="z01")
                nc.vector.tensor_scalar(
                    z01, accs[:, j, TC:TC + 1], SLOPE, bgz_sb[br][:, j:j + 1],
                    AL.mult, AL.add)
                nc.vector.tensor_scalar(
                    m_sb[:, j:j + 1], s4, 1.0 - SLOPE, z01, AL.mult, AL.add)

            # ---- g = leaky(mean @ Wf + bf)  (1/N folded into Wf host-side) ----
            gps = ps_sm.tile([128, 1], mybir.dt.float32, name=f"gps{br}", tag="sps")
            for kt in range(KT):
                nc.tensor.matmul(gps, lhsT=wf_sb[br][:, kt, :],
                                 rhs=m_sb[:, kt:kt + 1],
                                 start=(kt == 0), stop=(kt == KT - 1))
            vt = p_vec.tile([128, 1], f32, name=f"vt{br}", tag="vt")
            nc.scalar.activation(out=vt, in_=gps, func=AF.Identity, bias=bf_sb[br])
            wt = p_vec.tile([128, 1], f32, name=f"wt{br}", tag="wt")
            nc.vector.tensor_scalar_mul(wt, vt, SLOPE)
            gv = p_vec.tile([128, 1], f32, name=f"gv{br}", tag=f"gv{br}")
            nc.vector.tensor_tensor(gv, vt, wt, AL.max)
            g_vec.append(gv)

        # ---- head MLP: 256 -> 256 -> 64 -> 1, sigmoid ----
        xc1 = []
        for mb in range(2):
            xps = ps_sm.tile([128, 1], mybir.dt.float32, name=f"xps{mb}", tag="sps")
            for kt in range(2):
                nc.tensor.matmul(xps, lhsT=w1_sb[:, kt, mb * 128:(mb + 1) * 128],
                                 rhs=g_vec[kt], start=(kt == 0), stop=(kt == 1))
            vt = p_vec.tile([128, 1], f32, name=f"vh{mb}", tag="vt")
            nc.scalar.activation(out=vt, in_=xps, func=AF.Identity,
                                 bias=b1_sb[:, mb:mb + 1])
            wt = p_vec.tile([128, 1], f32, name=f"wh{mb}", tag="wt")
            nc.vector.tensor_scalar_mul(wt, vt, SLOPE)
            xv = p_vec.tile([128, 1], f32, name=f"xv{mb}", tag=f"xv{mb}")
            nc.vector.tensor_tensor(xv, vt, wt, AL.max)
            xc1.append(xv)

        x2ps = ps_sm.tile([128, 1], mybir.dt.float32, name="x2ps", tag="sps")
        for kt in range(2):
            nc.tensor.matmul(x2ps[:64], lhsT=w2_sb[:, kt, :], rhs=xc1[kt],
                             start=(kt == 0), stop=(kt == 1))
        v2 = p_vec.tile([64, 1], f32, name="v2", tag="v2")
        nc.scalar.activation(out=v2, in_=x2ps[:64], func=AF.Identity, bias=b2_sb)
        w2t = p_vec.tile([64, 1], f32, name="w2t", tag="w2t")
        nc.vector.tensor_scalar_mul(w2t, v2, SLOPE)
        xc2 = p_vec.tile([64, 1], f32, name="xc2", tag="xc2")
        nc.vector.tensor_tensor(xc2, v2, w2t, AL.max)

        ops_ = ps_sm.tile([1, 1], mybir.dt.float32, name="ops_", tag="sps")
        nc.tensor.matmul(ops_, lhsT=wo_sb, rhs=xc2, start=True, stop=True)
        osb = p_vec.tile([1, 1], f32, name="osb", tag="osb")
        nc.scalar.activation(out=osb, in_=ops_, func=AF.Sigmoid, bias=bo_sb)
        nc.sync.dma_start(out=out_d[:, :], in_=osb)

    nc.finalize()
    return nc


def _get_nc():
    global _NC
    if _NC is None:
        _NC = _build_program()
    return _NC


def _prep_branch(x, ei):
    """Host prep for one (graph, branch): x^T bf16 and the dense normalized
    adjacency, transposed, with col 2000 = row sums (gives sum_t Z)."""
    src = ei[0].astype(np.int64)
    tgt = ei[1].astype(np.int64)
    deg = (np.bincount(tgt, minlength=N) + 1).astype(np.float32)
    dinv = (1.0 / np.sqrt(deg)).astype(np.float32)
    at = np.zeros((NT, N + 1), np.float32)
    np.add.at(at, (src, tgt), dinv[src] * dinv[tgt])
    di = np.arange(N)
    at[di, di] += dinv * dinv
    at[:, N] = at[:, :N].sum(axis=1)         # col 2000 = row sums -> sum_t Z
    xt = np.ascontiguousarray(x.T).astype(_FP8 if FP8_MM1 else _BF16)
    return xt, at.astype(_FP8 if FP8_MM2 else _BF16)


def _make_in_maps(x1, ei1, x2, ei2, Wg1, bg1, Wf1, bf1, Wg2, bg2, Wf2, bf2,
                  W1, b1, W2, b2, Wo, bo):
    shared = {
        "wg1": np.ascontiguousarray(Wg1.astype(_FP8 if FP8_MM1 else _BF16)),
        "wg2": np.ascontiguousarray(Wg2.astype(_FP8 if FP8_MM1 else _BF16)),
        "wf1": np.ascontiguousarray((Wf1 / float(N)).astype(np.float32)),
        "wf2": np.ascontiguousarray((Wf2 / float(N)).astype(np.float32)),
        "bf1": bf1.reshape(D, 1).astype(np.float32),
        "bf2": bf2.reshape(D, 1).astype(np.float32),
        "bg1": np.ascontiguousarray(bg1.reshape(KT, 128).T.astype(np.float32)),
        "bg2": np.ascontiguousarray(bg2.reshape(KT, 128).T.astype(np.float32)),
        "w1": np.ascontiguousarray(W1.astype(np.float32)),
        "b1": np.ascontiguousarray(b1.reshape(2, 128).T.astype(np.float32)),
        "w2": np.ascontiguousarray(W2.astype(np.float32)),
        "b2": b2.reshape(64, 1).astype(np.float32),
        "wo": Wo.reshape(64, 1).astype(np.float32),
        "bo": bo.reshape(1, 1).astype(np.float32),
    }
    in_maps = []
    for b in range(B):
        m = dict(shared)
        m["xt1"], m["at1"] = _prep_branch(x1[b], ei1[b])
        m["xt2"], m["at2"] = _prep_branch(x2[b], ei2[b])
        in_maps.append(m)
    return in_maps


def kernel(**inputs):
    from concourse.bass_utils import run_bass_kernel_spmd

    nc = _get_nc()
    in_maps = _make_in_maps(**{k: np.asarray(v) for k, v in inputs.items()})
    res = run_bass_kernel_spmd(nc, in_maps, core_ids=list(range(B)))
    out = np.stack([res.results[c]["out"].reshape(1) for c in range(B)], axis=0)
    return out.astype(np.float32)


# revision 20
# speedup vs baseline: 1.9543x; 1.0523x over previous
"""Trainium2 Bass kernel for a 2-branch GCN siamese network (protein pairs).

Math per graph b (see reference):
    h  = leaky( A_norm @ (x @ Wg) + bg )        # GCNConv + LeakyReLU
    g  = leaky( mean_n(h) @ Wf + bf )
    xc = concat(g1, g2); 2-layer MLP + sigmoid -> scalar

Sharding: data-parallel over the batch of 8 graphs -> core b handles graph b
entirely (both branches + head) and emits a single scalar.

Device strategy:
  - A_norm is materialized dense (2048x2001, bf16) on the host, transposed,
    with the symmetric-norm coefficients folded in; column 2000 holds per-
    source row sums so Sum_t Z[j,t] falls out of the same matmuls.
  - MM1 (PE, bf16): H[n, j] = x @ Wg, accumulated in PSUM over 8 k-tiles.
  - MM2 (PE, bf16): Z^T[j, t] = sum_s H[s, j] * A^T[s, t], feature-major,
    K=80 partial tail tile so H's pad rows are never read.
  - leaky+mean pooling fused into ACT:  leaky(z+bg) = 0.01*(z+bg) +
    0.99*relu(z+bg); relu comes from ACT Relu with per-partition bias and
    accum_out, the linear term from the column-2000 sums.
  - Wf projection, head MLP and sigmoid run in fp32 on the PE (tiny).
"""

import os
import sys

import numpy as np

for _p in ("/opt/trn_rl_repo", "/root/.axon_site/_ro/trn_rl_repo"):
    if os.path.isdir(_p) and _p not in sys.path:
        sys.path.insert(0, _p)

import ml_dtypes

B, N, E, F, D = 8, 2000, 64000, 1024, 128
NT = 2048          # padded node count (sources)
KT = F // 128      # 8 k-tiles over the feature dim
NB = 16            # node blocks for MM1 (15 full + one of 80)
ST = NT // 128     # 16 source tiles for MM2 (last one K=80)
TC = 4             # target chunks for MM2: widths 512,512,512,465
WLAST = N - 3 * 512 + 1   # 465: 464 real targets + the col-2000 sum column
SLOPE = 0.01

_BF16 = ml_dtypes.bfloat16
_FP8 = ml_dtypes.float8_e4m3

# fp8e4m3 + DoubleRow for the big A^T matmul (MM2): ~1.7x fewer PE cycles.
# A entries are in [0.016, ~3] and H ~ N(0,1) — well inside e4m3 range; the
# per-element ~4% rounding washes out to ~1e-3 after mean-pooling over 2000
# nodes. Set False to fall back to bf16 (rel err ~7e-6).
FP8_MM2 = True
# fp8 + DoubleRow for x@Wg as well (numpy model: max batch rel err 1.8e-4).
FP8_MM1 = True

_NC = None


def _build_program():
    import concourse.bacc as bacc
    import concourse.mybir as mybir
    import concourse.tile as tile

    f32 = mybir.dt.float32
    bf16 = mybir.dt.bfloat16
    AF = mybir.ActivationFunctionType
    AL = mybir.AluOpType
    AX = mybir.AxisListType

    # Bacc (not plain Bass): its compile() runs generate_event_semaphores,
    # which splits multi-sem waits — walrus allows 1 sync wait per instruction.
    nc = bacc.Bacc()

    def ein(name, shape, dt):
        return nc.dram_tensor(name, shape, dt, kind="ExternalInput")

    f8 = mybir.dt.float8e4
    hdt = f8 if FP8_MM2 else bf16
    xdt = f8 if FP8_MM1 else bf16
    xt_d = [ein("xt1", [F, N], xdt), ein("xt2", [F, N], xdt)]
    wg_d = [ein("wg1", [F, F], xdt), ein("wg2", [F, F], xdt)]
    at_d = [ein("at1", [NT, N + 1], hdt), ein("at2", [NT, N + 1], hdt)]
    bg_d = [ein("bg1", [128, KT], f32), ein("bg2", [128, KT], f32)]
    wf_d = [ein("wf1", [F, D], f32), ein("wf2", [F, D], f32)]
    bf_d = [ein("bf1", [D, 1], f32), ein("bf2", [D, 1], f32)]
    w1_d = ein("w1", [2 * D, 256], f32)
    b1_d = ein("b1", [128, 2], f32)
    w2_d = ein("w2", [256, 64], f32)
    b2_d = ein("b2", [64, 1], f32)
    wo_d = ein("wo", [64, 1], f32)
    bo_d = ein("bo", [1, 1], f32)
    out_d = nc.dram_tensor("out", [1, 1], f32, kind="ExternalOutput")

    cw = [512, 512, 512, WLAST]          # MM2 chunk widths
    c0 = [0, 512, 1024, 1536]            # chunk column offsets in A^T

    with tile.TileContext(nc) as tc, \
            tc.tile_pool(name="p_xt", bufs=2) as p_xt, \
            tc.tile_pool(name="p_wg", bufs=2) as p_wg, \
            tc.tile_pool(name="p_h", bufs=2) as p_h, \
            tc.tile_pool(name="p_at", bufs=2) as p_at, \
            tc.tile_pool(name="p_c", bufs=1) as p_c, \
            tc.tile_pool(name="p_scr", bufs=3) as p_scr, \
            tc.tile_pool(name="p_vec", bufs=2) as p_vec, \
            tc.tile_pool(name="ps_mm1", bufs=4, space="PSUM") as ps_mm1, \
            tc.tile_pool(name="ps_mm2", bufs=2, space="PSUM") as ps_mm2, \
            tc.tile_pool(name="ps_sm", bufs=2, space="PSUM") as ps_sm:

        # ================= DMA issue order is the critical path =============
        # Interleave wg1 k-tiles with xt1 node-blocks so MM1(b0) starts ~2us
        # in; then bgr1 + the first two A^T chunks; bulk/late consts after.
        wg_sb = [p_wg.tile([128, KT, F], xdt, name=f"wg_sb{br}", tag="wg")
                 for br in range(2)]
        xt_sb = [p_xt.tile([128, KT, N], xdt, name=f"xt_sb{br}", tag="xt")
                 for br in range(2)]
        wgr = [wg_d[br][:, :].rearrange("(kt p) j -> p kt j", p=128)
               for br in range(2)]
        xtr = [xt_d[br][:, :].rearrange("(kt p) n -> p kt n", p=128)
               for br in range(2)]

        def load_xt(br, q):
            n0, n1 = q * 500, min(N, q * 500 + 500)
            nc.sync.dma_start(out=xt_sb[br][:, :, n0:n1], in_=xtr[br][:, :, n0:n1])

        def load_wg(br, h):
            nc.sync.dma_start(out=wg_sb[br][:, 4 * h:4 * h + 4, :],
                              in_=wgr[br][:, 4 * h:4 * h + 4, :])

        load_wg(0, 0)
        load_xt(0, 0)
        load_wg(0, 1)
        for q in range(1, 4):
            load_xt(0, q)

        bgr_sb, bgz_sb = [], []
        for br in range(2):
            bgr_t = p_c.tile([128, KT], f32, name=f"bgr_sb{br}", tag=f"bgr{br}")
            bgz_t = p_c.tile([128, KT], f32, name=f"bgz_sb{br}", tag=f"bgz{br}")
            bgr_sb.append(bgr_t)
            bgz_sb.append(bgz_t)
        nc.sync.dma_start(out=bgr_sb[0][:], in_=bg_d[0][:, :])

        # A^T chunk tiles (2 slots; pre-issue branch 0 chunks 0 and 1)
        atr = [at_d[br][:, :].rearrange("(so p) t -> p so t", p=128)
               for br in range(2)]
        # fixed 512-wide tiles so the s-dim stride stays 16-aligned (DR req)
        at_sb = [[p_at.tile([128, ST, 512], hdt,
                            name=f"at_sb{br}_{tcx}", tag="at")
                  for tcx in range(TC)] for br in range(2)]

        def load_at(br, tcx):
            nc.sync.dma_start(
                out=at_sb[br][tcx][:, :, :cw[tcx]],
                in_=atr[br][:, :, c0[tcx]:c0[tcx] + cw[tcx]])

        load_at(0, 0)
        load_at(0, 1)

        # remaining (late-needed) constants
        load_wg(1, 0)
        load_wg(1, 1)
        nc.sync.dma_start(out=bgr_sb[1][:], in_=bg_d[1][:, :])
        wf_sb, bf_sb = [], []
        for br in range(2):
            wf_t = p_c.tile([128, KT, D], f32, name=f"wf_sb{br}", tag=f"wf{br}")
            nc.sync.dma_start(
                out=wf_t[:], in_=wf_d[br][:, :].rearrange("(kt p) d -> p kt d", p=128))
            wf_sb.append(wf_t)
            bf_t = p_c.tile([D, 1], f32, name=f"bf_sb{br}", tag=f"bf{br}")
            nc.sync.dma_start(out=bf_t[:], in_=bf_d[br][:, :])
            bf_sb.append(bf_t)
        w1_sb = p_c.tile([128, 2, 256], f32, name="w1_sb", tag="w1")
        nc.sync.dma_start(
            out=w1_sb[:], in_=w1_d[:, :].rearrange("(kt p) m -> p kt m", p=128))
        b1_sb = p_c.tile([128, 2], f32, name="b1_sb", tag="b1")
        nc.sync.dma_start(out=b1_sb[:], in_=b1_d[:, :])
        w2_sb = p_c.tile([128, 2, 64], f32, name="w2_sb", tag="w2")
        nc.sync.dma_start(
            out=w2_sb[:], in_=w2_d[:, :].rearrange("(kt p) m -> p kt m", p=128))
        b2_sb = p_c.tile([64, 1], f32, name="b2_sb", tag="b2")
        nc.sync.dma_start(out=b2_sb[:], in_=b2_d[:, :])
        wo_sb = p_c.tile([64, 1], f32, name="wo_sb", tag="wo")
        nc.sync.dma_start(out=wo_sb[:], in_=wo_d[:, :])
        bo_sb = p_c.tile([1, 1], f32, name="bo_sb", tag="bo")
        nc.sync.dma_start(out=bo_sb[:], in_=bo_d[:, :])

        for br in range(2):
            nc.vector.tensor_scalar_mul(bgz_sb[br], bgr_sb[br], SLOPE * float(N))

        # ========================== compute ================================
        g_vec = []
        for br in range(2):
            if br == 1:
                for q in range(4):
                    load_xt(1, q)

            h_sb = p_h.tile([128, ST, F], hdt, name=f"h_sb{br}", tag="h")
            if FP8_MM2:
                # DoubleRow pairs contract the full 16 s-tiles, so H's pad
                # rows (2000..2047) must be zero, not garbage.
                nc.vector.memset(h_sb[64:128, 15, :], 0.0)

            # ---- MM1: H[n, j] = x @ Wg ----
            for nb in range(NB):
                n0 = nb * 128
                m = min(128, N - n0)
                pt = [ps_mm1.tile([128, 512], mybir.dt.float32,
                                  name=f"mm1ps_{br}_{nb}_{jh}", tag="mm1ps")
                      for jh in range(2)]
                if FP8_MM1:
                    for kp in range(KT // 2):
                        for jh in range(2):
                            nc.tensor.matmul(
                                pt[jh][:m, :],
                                lhsT=xt_sb[br][:, 2 * kp:2 * kp + 2, n0:n0 + m],
                                rhs=wg_sb[br][:, 2 * kp:2 * kp + 2,
                                              jh * 512:(jh + 1) * 512],
                                start=(kp == 0), stop=(kp == KT // 2 - 1),
                                perf_mode=mybir.MatmulPerfMode.DoubleRow)
                else:
                    for kt in range(KT):
                        for jh in range(2):
                            nc.tensor.matmul(
                                pt[jh][:m, :],
                                lhsT=xt_sb[br][:, kt, n0:n0 + m],
                                rhs=wg_sb[br][:, kt, jh * 512:(jh + 1) * 512],
                                start=(kt == 0), stop=(kt == KT - 1))
                for jh in range(2):
                    nc.vector.tensor_copy(
                        out=h_sb[:m, nb, jh * 512:(jh + 1) * 512], in_=pt[jh][:m, :])

            # ---- MM2: Z^T[j, t] = sum_s H[s, j] A^T[s, t]; fused pooling ----
            accs = p_vec.tile([128, KT, TC + 1], f32, name=f"accs{br}", tag="accs")
            for tcx in range(TC):
                if br == 1:
                    load_at(1, tcx)
                elif tcx >= 2:
                    load_at(0, tcx)
                at_t = at_sb[br][tcx]
                for j in range(KT):
                    zps = ps_mm2.tile([128, 512], mybir.dt.float32,
                                      name=f"mm2ps_{br}_{tcx}_{j}", tag="mm2ps")
                    if FP8_MM2:
                        for sp in range(ST // 2):
                            nc.tensor.matmul(
                                zps[:, :cw[tcx]],
                                lhsT=h_sb[:, 2 * sp:2 * sp + 2,
                                          j * 128:(j + 1) * 128],
                                rhs=at_t[:, 2 * sp:2 * sp + 2, :cw[tcx]],
                                start=(sp == 0), stop=(sp == ST // 2 - 1),
                                perf_mode=mybir.MatmulPerfMode.DoubleRow)
                    else:
                        for s in range(ST):
                            kp = 128 if s < ST - 1 else N - (ST - 1) * 128
                            nc.tensor.matmul(
                                zps[:, :cw[tcx]],
                                lhsT=h_sb[:kp, s, j * 128:(j + 1) * 128],
                                rhs=at_t[:kp, s, :cw[tcx]],
                                start=(s == 0), stop=(s == ST - 1))
                    w = 512 if tcx < TC - 1 else WLAST - 1  # 464 real targets
                    scr = p_scr.tile([128, 512], bf16,
                                     name=f"scr_{br}_{tcx}_{j}", tag="scr")
                    nc.scalar.activation(
                        out=scr[:, :w], in_=zps[:, :w], func=AF.Relu,
                        bias=bgr_sb[br][:, j:j + 1],
                        accum_out=accs[:, j, tcx:tcx + 1])
                    if tcx == TC - 1:
                        # column 2000 of A^T carries sum_t Z[j, t]
                        nc.vector.tensor_copy(
                            out=accs[:, j, TC:TC + 1],
                            in_=zps[:, WLAST - 1:WLAST])

            # ---- pooled vector m[j] = 0.99*sum(relu) + 0.01*(sum(z)+N*bg) ----
            m_sb = p_vec.tile([128, KT], f32, name=f"m_sb{br}", tag="m")
            for j in range(KT):
                s4 = p_vec.tile([128, 1], f32, name=f"s4_{br}_{j}", tag="s4")
                nc.vector.tensor_reduce(s4, accs[:, j, 0:TC], AX.X, AL.add)
                z01 = p_vec.tile([128, 1], f32, name=f"z01_{br}_{j}", tag="z01")
                nc.vector.tensor_scalar(
                    z01, accs[:, j, TC:TC + 1], SLOPE, bgz_sb[br][:, j:j + 1],
                    AL.mult, AL.add)
                nc.vector.tensor_scalar(
                    m_sb[:, j:j + 1], s4, 1.0 - SLOPE, z01, AL.mult, AL.add)

            # ---- g = leaky(mean @ Wf + bf)  (1/N folded into Wf host-side) ----
            gps = ps_sm.tile([128, 1], mybir.dt.float32, name=f"gps{br}", tag="sps")
            for kt in range(KT):
                nc.tensor.matmul(gps, lhsT=wf_sb[br][:, kt, :],
                                 rhs=m_sb[:, kt:kt + 1],
                                 start=(kt == 0), stop=(kt == KT - 1))
            vt = p_vec.tile([128, 1], f32, name=f"vt{br}", tag="vt")
            nc.scalar.activation(out=vt, in_=gps, func=AF.Identity, bias=bf_sb[br])
            wt = p_vec.tile([128, 1], f32, name=f"wt{br}", tag="wt")
            nc.vector.tensor_scalar_mul(wt, vt, SLOPE)
            gv = p_vec.tile([128, 1], f32, name=f"gv{br}", tag=f"gv{br}")
            nc.vector.tensor_tensor(gv, vt, wt, AL.max)
            g_vec.append(gv)

        # ---- head MLP: 256 -> 256 -> 64 -> 1, sigmoid ----
        xc1 = []
        for mb in range(2):
            xps = ps_sm.tile([128, 1], mybir.dt.float32, name=f"xps{mb}", tag="sps")
            for kt in range(2):
                nc.tensor.matmul(xps, lhsT=w1_sb[:, kt, mb * 128:(mb + 1) * 128],
                                 rhs=g_vec[kt], start=(kt == 0), stop=(kt == 1))
            vt = p_vec.tile([128, 1], f32, name=f"vh{mb}", tag="vt")
            nc.scalar.activation(out=vt, in_=xps, func=AF.Identity,
                                 bias=b1_sb[:, mb:mb + 1])
            wt = p_vec.tile([128, 1], f32, name=f"wh{mb}", tag="wt")
            nc.vector.tensor_scalar_mul(wt, vt, SLOPE)
            xv = p_vec.tile([128, 1], f32, name=f"xv{mb}", tag=f"xv{mb}")
            nc.vector.tensor_tensor(xv, vt, wt, AL.max)
            xc1.append(xv)

        x2ps = ps_sm.tile([128, 1], mybir.dt.float32, name="x2ps", tag="sps")
        for kt in range(2):
            nc.tensor.matmul(x2ps[:64], lhsT=w2_sb[:, kt, :], rhs=xc1[kt],
                             start=(kt == 0), stop=(kt == 1))
        v2 = p_vec.tile([64, 1], f32, name="v2", tag="v2")
        nc.scalar.activation(out=v2, in_=x2ps[:64], func=AF.Identity, bias=b2_sb)
        w2t = p_vec.tile([64, 1], f32, name="w2t", tag="w2t")
        nc.vector.tensor_scalar_mul(w2t, v2, SLOPE)
        xc2 = p_vec.tile([64, 1], f32, name="xc2", tag="xc2")
        nc.vector.tensor_tensor(xc2, v2, w2t, AL.max)

        ops_ = ps_sm.tile([1, 1], mybir.dt.float32, name="ops_", tag="sps")
        nc.tensor.matmul(ops_, lhsT=wo_sb, rhs=xc2, start=True, stop=True)
        osb = p_vec.tile([1, 1], f32, name="osb", tag="osb")
        nc.scalar.activation(out=osb, in_=ops_, func=AF.Sigmoid, bias=bo_sb)
        nc.sync.dma_start(out=out_d[:, :], in_=osb)

    nc.finalize()
    return nc


def _get_nc():
    global _NC
    if _NC is None:
        _NC = _build_program()
    return _NC


def _prep_branch(x, ei):
    """Host prep for one (graph, branch): x^T bf16 and the dense normalized
    adjacency, transposed, with col 2000 = row sums (gives sum_t Z)."""
    src = ei[0].astype(np.int64)
    tgt = ei[1].astype(np.int64)
    deg = (np.bincount(tgt, minlength=N) + 1).astype(np.float32)
    dinv = (1.0 / np.sqrt(deg)).astype(np.float32)
    at = np.zeros((NT, N + 1), np.float32)
    np.add.at(at, (src, tgt), dinv[src] * dinv[tgt])
    di = np.arange(N)
    at[di, di] += dinv * dinv
    at[:, N] = at[:, :N].sum(axis=1)         # col 2000 = row sums -> sum_t Z
    xt = np.ascontiguousarray(x.T).astype(_FP8 if FP8_MM1 else _BF16)
    return xt, at.astype(_FP8 if FP8_MM2 else _BF16)


def _make_in_maps(x1, ei1, x2, ei2, Wg1, bg1, Wf1, bf1, Wg2, bg2, Wf2, bf2,
                  W1, b1, W2, b2, Wo, bo):
    shared = {
        "wg1": np.ascontiguousarray(Wg1.astype(_FP8 if FP8_MM1 else _BF16)),
        "wg2": np.ascontiguousarray(Wg2.astype(_FP8 if FP8_MM1 else _BF16)),
        "wf1": np.ascontiguousarray((Wf1 / float(N)).astype(np.float32)),
        "wf2": np.ascontiguousarray((Wf2 / float(N)).astype(np.float32)),
        "bf1": bf1.reshape(D, 1).astype(np.float32),
        "bf2": bf2.reshape(D, 1).astype(np.float32),
        "bg1": np.ascontiguousarray(bg1.reshape(KT, 128).T.astype(np.float32)),
        "bg2": np.ascontiguousarray(bg2.reshape(KT, 128).T.astype(np.float32)),
        "w1": np.ascontiguousarray(W1.astype(np.float32)),
        "b1": np.ascontiguousarray(b1.reshape(2, 128).T.astype(np.float32)),
        "w2": np.ascontiguousarray(W2.astype(np.float32)),
        "b2": b2.reshape(64, 1).astype(np.float32),
        "wo": Wo.reshape(64, 1).astype(np.float32),
        "bo": bo.reshape(1, 1).astype(np.float32),
    }
    in_maps = []
    for b in range(B):
        m = dict(shared)
        m["xt1"], m["at1"] = _prep_branch(x1[b], ei1[b])
        m["xt2"], m["at2"] = _prep_branch(x2[b], ei2[b])
        in_maps.append(m)
    return in_maps


def kernel(**inputs):
    from concourse.bass_utils import run_bass_kernel_spmd

    nc = _get_nc()
    in_maps = _make_in_maps(**{k: np.asarray(v) for k, v in inputs.items()})
    res = run_bass_kernel_spmd(nc, in_maps, core_ids=list(range(B)))
    out = np.stack([res.results[c]["out"].reshape(1) for c in range(B)], axis=0)
    return out.astype(np.float32)


# revision 21
# speedup vs baseline: 1.9645x; 1.0052x over previous
"""Trainium2 Bass kernel for a 2-branch GCN siamese network (protein pairs).

Math per graph b (see reference):
    h  = leaky( A_norm @ (x @ Wg) + bg )        # GCNConv + LeakyReLU
    g  = leaky( mean_n(h) @ Wf + bf )
    xc = concat(g1, g2); 2-layer MLP + sigmoid -> scalar

Sharding: data-parallel over the batch of 8 graphs -> core b handles graph b
entirely (both branches + head) and emits a single scalar.

Device strategy:
  - A_norm is materialized dense (2048x2001, bf16) on the host, transposed,
    with the symmetric-norm coefficients folded in; column 2000 holds per-
    source row sums so Sum_t Z[j,t] falls out of the same matmuls.
  - MM1 (PE, bf16): H[n, j] = x @ Wg, accumulated in PSUM over 8 k-tiles.
  - MM2 (PE, bf16): Z^T[j, t] = sum_s H[s, j] * A^T[s, t], feature-major,
    K=80 partial tail tile so H's pad rows are never read.
  - leaky+mean pooling fused into ACT:  leaky(z+bg) = 0.01*(z+bg) +
    0.99*relu(z+bg); relu comes from ACT Relu with per-partition bias and
    accum_out, the linear term from the column-2000 sums.
  - Wf projection, head MLP and sigmoid run in fp32 on the PE (tiny).
"""

import os
import sys

import numpy as np

for _p in ("/opt/trn_rl_repo", "/root/.axon_site/_ro/trn_rl_repo"):
    if os.path.isdir(_p) and _p not in sys.path:
        sys.path.insert(0, _p)

import ml_dtypes

B, N, E, F, D = 8, 2000, 64000, 1024, 128
NT = 2048          # padded node count (sources)
KT = F // 128      # 8 k-tiles over the feature dim
NB = 16            # node blocks for MM1 (15 full + one of 80)
ST = NT // 128     # 16 source tiles for MM2 (last one K=80)
TC = 4             # target chunks for MM2: widths 512,512,512,465
WLAST = N - 3 * 512 + 1   # 465: 464 real targets + the col-2000 sum column
SLOPE = 0.01

_BF16 = ml_dtypes.bfloat16
_FP8 = ml_dtypes.float8_e4m3

# fp8e4m3 + DoubleRow for the big A^T matmul (MM2): ~1.7x fewer PE cycles.
# A entries are in [0.016, ~3] and H ~ N(0,1) — well inside e4m3 range; the
# per-element ~4% rounding washes out to ~1e-3 after mean-pooling over 2000
# nodes. Set False to fall back to bf16 (rel err ~7e-6).
FP8_MM2 = True
# fp8 + DoubleRow for x@Wg as well (numpy model: max batch rel err 1.8e-4).
FP8_MM1 = True

_NC = None


def _build_program():
    import concourse.bacc as bacc
    import concourse.mybir as mybir
    import concourse.tile as tile

    f32 = mybir.dt.float32
    bf16 = mybir.dt.bfloat16
    AF = mybir.ActivationFunctionType
    AL = mybir.AluOpType
    AX = mybir.AxisListType

    # Bacc (not plain Bass): its compile() runs generate_event_semaphores,
    # which splits multi-sem waits — walrus allows 1 sync wait per instruction.
    nc = bacc.Bacc()

    def ein(name, shape, dt):
        return nc.dram_tensor(name, shape, dt, kind="ExternalInput")

    f8 = mybir.dt.float8e4
    hdt = f8 if FP8_MM2 else bf16
    xdt = f8 if FP8_MM1 else bf16
    xt_d = [ein("xt1", [F, N], xdt), ein("xt2", [F, N], xdt)]
    wg_d = [ein("wg1", [F, F], xdt), ein("wg2", [F, F], xdt)]
    at_d = [ein("at1", [NT, N + 1], hdt), ein("at2", [NT, N + 1], hdt)]
    bg_d = [ein("bg1", [128, KT], f32), ein("bg2", [128, KT], f32)]
    wf_d = [ein("wf1", [F, D], f32), ein("wf2", [F, D], f32)]
    bf_d = [ein("bf1", [D, 1], f32), ein("bf2", [D, 1], f32)]
    w1_d = ein("w1", [2 * D, 256], f32)
    b1_d = ein("b1", [128, 2], f32)
    w2_d = ein("w2", [256, 64], f32)
    b2_d = ein("b2", [64, 1], f32)
    wo_d = ein("wo", [64, 1], f32)
    bo_d = ein("bo", [1, 1], f32)
    out_d = nc.dram_tensor("out", [1, 1], f32, kind="ExternalOutput")

    cw = [512, 512, 512, WLAST]          # MM2 chunk widths
    c0 = [0, 512, 1024, 1536]            # chunk column offsets in A^T

    with tile.TileContext(nc) as tc, \
            tc.tile_pool(name="p_xt", bufs=2) as p_xt, \
            tc.tile_pool(name="p_wg", bufs=2) as p_wg, \
            tc.tile_pool(name="p_h", bufs=2) as p_h, \
            tc.tile_pool(name="p_at", bufs=2) as p_at, \
            tc.tile_pool(name="p_c", bufs=1) as p_c, \
            tc.tile_pool(name="p_scr", bufs=3) as p_scr, \
            tc.tile_pool(name="p_vec", bufs=2) as p_vec, \
            tc.tile_pool(name="ps_mm1", bufs=4, space="PSUM") as ps_mm1, \
            tc.tile_pool(name="ps_mm2", bufs=2, space="PSUM") as ps_mm2, \
            tc.tile_pool(name="ps_sm", bufs=2, space="PSUM") as ps_sm:

        # ================= DMA issue order is the critical path =============
        # Interleave wg1 k-tiles with xt1 node-blocks so MM1(b0) starts ~2us
        # in; then bgr1 + the first two A^T chunks; bulk/late consts after.
        wg_sb = [p_wg.tile([128, KT, F], xdt, name=f"wg_sb{br}", tag="wg")
                 for br in range(2)]
        xt_sb = [p_xt.tile([128, KT, N], xdt, name=f"xt_sb{br}", tag="xt")
                 for br in range(2)]
        wgr = [wg_d[br][:, :].rearrange("(kt p) j -> p kt j", p=128)
               for br in range(2)]
        xtr = [xt_d[br][:, :].rearrange("(kt p) n -> p kt n", p=128)
               for br in range(2)]

        def load_xt(br, q):
            n0, n1 = q * 500, min(N, q * 500 + 500)
            nc.sync.dma_start(out=xt_sb[br][:, :, n0:n1], in_=xtr[br][:, :, n0:n1])

        def load_wg(br, h):
            nc.sync.dma_start(out=wg_sb[br][:, 4 * h:4 * h + 4, :],
                              in_=wgr[br][:, 4 * h:4 * h + 4, :])

        # tiny first pieces so MM1(b0, nb0) can start right after the entry
        # barrier; then bulk pieces stream behind the PE
        nc.sync.dma_start(out=wg_sb[0][:, 0:2, :], in_=wgr[0][:, 0:2, :])
        nc.sync.dma_start(out=xt_sb[0][:, :, 0:128], in_=xtr[0][:, :, 0:128])
        nc.sync.dma_start(out=wg_sb[0][:, 2:4, :], in_=wgr[0][:, 2:4, :])
        nc.sync.dma_start(out=xt_sb[0][:, :, 128:500], in_=xtr[0][:, :, 128:500])
        load_wg(0, 1)
        for q in range(1, 4):
            load_xt(0, q)

        bgr_sb, bgz_sb = [], []
        for br in range(2):
            bgr_t = p_c.tile([128, KT], f32, name=f"bgr_sb{br}", tag=f"bgr{br}")
            bgz_t = p_c.tile([128, KT], f32, name=f"bgz_sb{br}", tag=f"bgz{br}")
            bgr_sb.append(bgr_t)
            bgz_sb.append(bgz_t)
        nc.sync.dma_start(out=bgr_sb[0][:], in_=bg_d[0][:, :])

        # A^T chunk tiles (2 slots; pre-issue branch 0 chunks 0 and 1)
        atr = [at_d[br][:, :].rearrange("(so p) t -> p so t", p=128)
               for br in range(2)]
        # fixed 512-wide tiles so the s-dim stride stays 16-aligned (DR req)
        at_sb = [[p_at.tile([128, ST, 512], hdt,
                            name=f"at_sb{br}_{tcx}", tag="at")
                  for tcx in range(TC)] for br in range(2)]

        def load_at(br, tcx):
            nc.sync.dma_start(
                out=at_sb[br][tcx][:, :, :cw[tcx]],
                in_=atr[br][:, :, c0[tcx]:c0[tcx] + cw[tcx]])

        load_at(0, 0)
        load_at(0, 1)

        # remaining (late-needed) constants
        load_wg(1, 0)
        load_wg(1, 1)
        nc.sync.dma_start(out=bgr_sb[1][:], in_=bg_d[1][:, :])
        wf_sb, bf_sb = [], []
        for br in range(2):
            wf_t = p_c.tile([128, KT, D], f32, name=f"wf_sb{br}", tag=f"wf{br}")
            nc.sync.dma_start(
                out=wf_t[:], in_=wf_d[br][:, :].rearrange("(kt p) d -> p kt d", p=128))
            wf_sb.append(wf_t)
            bf_t = p_c.tile([D, 1], f32, name=f"bf_sb{br}", tag=f"bf{br}")
            nc.sync.dma_start(out=bf_t[:], in_=bf_d[br][:, :])
            bf_sb.append(bf_t)
        w1_sb = p_c.tile([128, 2, 256], f32, name="w1_sb", tag="w1")
        nc.sync.dma_start(
            out=w1_sb[:], in_=w1_d[:, :].rearrange("(kt p) m -> p kt m", p=128))
        b1_sb = p_c.tile([128, 2], f32, name="b1_sb", tag="b1")
        nc.sync.dma_start(out=b1_sb[:], in_=b1_d[:, :])
        w2_sb = p_c.tile([128, 2, 64], f32, name="w2_sb", tag="w2")
        nc.sync.dma_start(
            out=w2_sb[:], in_=w2_d[:, :].rearrange("(kt p) m -> p kt m", p=128))
        b2_sb = p_c.tile([64, 1], f32, name="b2_sb", tag="b2")
        nc.sync.dma_start(out=b2_sb[:], in_=b2_d[:, :])
        wo_sb = p_c.tile([64, 1], f32, name="wo_sb", tag="wo")
        nc.sync.dma_start(out=wo_sb[:], in_=wo_d[:, :])
        bo_sb = p_c.tile([1, 1], f32, name="bo_sb", tag="bo")
        nc.sync.dma_start(out=bo_sb[:], in_=bo_d[:, :])

        for br in range(2):
            nc.vector.tensor_scalar_mul(bgz_sb[br], bgr_sb[br], SLOPE * float(N))

        # ========================== compute ================================
        g_vec = []
        for br in range(2):
            if br == 1:
                for q in range(4):
                    load_xt(1, q)

            h_sb = p_h.tile([128, ST, F], hdt, name=f"h_sb{br}", tag="h")
            if FP8_MM2:
                # DoubleRow pairs contract the full 16 s-tiles, so H's pad
                # rows (2000..2047) must be zero, not garbage.
                nc.vector.memset(h_sb[64:128, 15, :], 0.0)

            # ---- MM1: H[n, j] = x @ Wg ----
            for nb in range(NB):
                n0 = nb * 128
                m = min(128, N - n0)
                pt = [ps_mm1.tile([128, 512], mybir.dt.float32,
                                  name=f"mm1ps_{br}_{nb}_{jh}", tag="mm1ps")
                      for jh in range(2)]
                if FP8_MM1:
                    for kp in range(KT // 2):
                        for jh in range(2):
                            nc.tensor.matmul(
                                pt[jh][:m, :],
                                lhsT=xt_sb[br][:, 2 * kp:2 * kp + 2, n0:n0 + m],
                                rhs=wg_sb[br][:, 2 * kp:2 * kp + 2,
                                              jh * 512:(jh + 1) * 512],
                                start=(kp == 0), stop=(kp == KT // 2 - 1),
                                perf_mode=mybir.MatmulPerfMode.DoubleRow)
                else:
                    for kt in range(KT):
                        for jh in range(2):
                            nc.tensor.matmul(
                                pt[jh][:m, :],
                                lhsT=xt_sb[br][:, kt, n0:n0 + m],
                                rhs=wg_sb[br][:, kt, jh * 512:(jh + 1) * 512],
                                start=(kt == 0), stop=(kt == KT - 1))
                for jh in range(2):
                    nc.vector.tensor_copy(
                        out=h_sb[:m, nb, jh * 512:(jh + 1) * 512], in_=pt[jh][:m, :])

            # ---- MM2: Z^T[j, t] = sum_s H[s, j] A^T[s, t]; fused pooling ----
            accs = p_vec.tile([128, KT, TC + 1], f32, name=f"accs{br}", tag="accs")
            for tcx in range(TC):
                if br == 1:
                    load_at(1, tcx)
                elif tcx >= 2:
                    load_at(0, tcx)
                at_t = at_sb[br][tcx]
                for j in range(KT):
                    zps = ps_mm2.tile([128, 512], mybir.dt.float32,
                                      name=f"mm2ps_{br}_{tcx}_{j}", tag="mm2ps")
                    if FP8_MM2:
                        for sp in range(ST // 2):
                            nc.tensor.matmul(
                                zps[:, :cw[tcx]],
                                lhsT=h_sb[:, 2 * sp:2 * sp + 2,
                                          j * 128:(j + 1) * 128],
                                rhs=at_t[:, 2 * sp:2 * sp + 2, :cw[tcx]],
                                start=(sp == 0), stop=(sp == ST // 2 - 1),
                                perf_mode=mybir.MatmulPerfMode.DoubleRow)
                    else:
                        for s in range(ST):
                            kp = 128 if s < ST - 1 else N - (ST - 1) * 128
                            nc.tensor.matmul(
                                zps[:, :cw[tcx]],
                                lhsT=h_sb[:kp, s, j * 128:(j + 1) * 128],
                                rhs=at_t[:kp, s, :cw[tcx]],
                                start=(s == 0), stop=(s == ST - 1))
                    w = 512 if tcx < TC - 1 else WLAST - 1  # 464 real targets
                    scr = p_scr.tile([128, 512], bf16,
                                     name=f"scr_{br}_{tcx}_{j}", tag="scr")
                    nc.scalar.activation(
                        out=scr[:, :w], in_=zps[:, :w], func=AF.Relu,
                        bias=bgr_sb[br][:, j:j + 1],
                        accum_out=accs[:, j, tcx:tcx + 1])
                    if tcx == TC - 1:
                        # column 2000 of A^T carries sum_t Z[j, t]
                        nc.vector.tensor_copy(
                            out=accs[:, j, TC:TC + 1],
                            in_=zps[:, WLAST - 1:WLAST])

            # ---- pooled vector m[j] = 0.99*sum(relu) + 0.01*(sum(z)+N*bg) ----
            m_sb = p_vec.tile([128, KT], f32, name=f"m_sb{br}", tag="m")
            for j in range(KT):
                s4 = p_vec.tile([128, 1], f32, name=f"s4_{br}_{j}", tag="s4")
                nc.vector.tensor_reduce(s4, accs[:, j, 0:TC], AX.X, AL.add)
                z01 = p_vec.tile([128, 1], f32, name=f"z01_{br}_{j}", tag="z01")
                nc.vector.tensor_scalar(
                    z01, accs[:, j, TC:TC + 1], SLOPE, bgz_sb[br][:, j:j + 1],
                    AL.mult, AL.add)
                nc.vector.tensor_scalar(
                    m_sb[:, j:j + 1], s4, 1.0 - SLOPE, z01, AL.mult, AL.add)

            # ---- g = leaky(mean @ Wf + bf)  (1/N folded into Wf host-side) ----
            gps = ps_sm.tile([128, 1], mybir.dt.float32, name=f"gps{br}", tag="sps")
            for kt in range(KT):
                nc.tensor.matmul(gps, lhsT=wf_sb[br][:, kt, :],
                                 rhs=m_sb[:, kt:kt + 1],
                                 start=(kt == 0), stop=(kt == KT - 1))
            vt = p_vec.tile([128, 1], f32, name=f"vt{br}", tag="vt")
            nc.scalar.activation(out=vt, in_=gps, func=AF.Identity, bias=bf_sb[br])
            wt = p_vec.tile([128, 1], f32, name=f"wt{br}", tag="wt")
            nc.vector.tensor_scalar_mul(wt, vt, SLOPE)
            gv = p_vec.tile([128, 1], f32, name=f"gv{br}", tag=f"gv{br}")
            nc.vector.tensor_tensor(gv, vt, wt, AL.max)
            g_vec.append(gv)

        # ---- head MLP: 256 -> 256 -> 64 -> 1, sigmoid ----
        xc1 = []
        for mb in range(2):
            xps = ps_sm.tile([128, 1], mybir.dt.float32, name=f"xps{mb}", tag="sps")
            for kt in range(2):
                nc.tensor.matmul(xps, lhsT=w1_sb[:, kt, mb * 128:(mb + 1) * 128],
                                 rhs=g_vec[kt], start=(kt == 0), stop=(kt == 1))
            vt = p_vec.tile([128, 1], f32, name=f"vh{mb}", tag="vt")
            nc.scalar.activation(out=vt, in_=xps, func=AF.Identity,
                                 bias=b1_sb[:, mb:mb + 1])
            wt = p_vec.tile([128, 1], f32, name=f"wh{mb}", tag="wt")
            nc.vector.tensor_scalar_mul(wt, vt, SLOPE)
            xv = p_vec.tile([128, 1], f32, name=f"xv{mb}", tag=f"xv{mb}")
            nc.vector.tensor_tensor(xv, vt, wt, AL.max)
            xc1.append(xv)

        x2ps = ps_sm.tile([128, 1], mybir.dt.float32, name="x2ps", tag="sps")
        for kt in range(2):
            nc.tensor.matmul(x2ps[:64], lhsT=w2_sb[:, kt, :], rhs=xc1[kt],
                             start=(kt == 0), stop=(kt == 1))
        v2 = p_vec.tile([64, 1], f32, name="v2", tag="v2")
        nc.scalar.activation(out=v2, in_=x2ps[:64], func=AF.Identity, bias=b2_sb)
        w2t = p_vec.tile([64, 1], f32, name="w2t", tag="w2t")
        nc.vector.tensor_scalar_mul(w2t, v2, SLOPE)
        xc2 = p_vec.tile([64, 1], f32, name="xc2", tag="xc2")
        nc.vector.tensor_tensor(xc2, v2, w2t, AL.max)

        ops_ = ps_sm.tile([1, 1], mybir.dt.float32, name="ops_", tag="sps")
        nc.tensor.matmul(ops_, lhsT=wo_sb, rhs=xc2, start=True, stop=True)
        osb = p_vec.tile([1, 1], f32, name="osb", tag="osb")
        nc.scalar.activation(out=osb, in_=ops_, func=AF.Sigmoid, bias=bo_sb)
        nc.sync.dma_start(out=out_d[:, :], in_=osb)

    nc.finalize()
    return nc


def _get_nc():
    global _NC
    if _NC is None:
        _NC = _build_program()
    return _NC


def _prep_branch(x, ei):
    """Host prep for one (graph, branch): x^T bf16 and the dense normalized
    adjacency, transposed, with col 2000 = row sums (gives sum_t Z)."""
    src = ei[0].astype(np.int64)
    tgt = ei[1].astype(np.int64)
    deg = (np.bincount(tgt, minlength=N) + 1).astype(np.float32)
    dinv = (1.0 / np.sqrt(deg)).astype(np.float32)
    at = np.zeros((NT, N + 1), np.float32)
    np.add.at(at, (src, tgt), dinv[src] * dinv[tgt])
    di = np.arange(N)
    at[di, di] += dinv * dinv
    at[:, N] = at[:, :N].sum(axis=1)         # col 2000 = row sums -> sum_t Z
    xt = np.ascontiguousarray(x.T).astype(_FP8 if FP8_MM1 else _BF16)
    return xt, at.astype(_FP8 if FP8_MM2 else _BF16)


def _make_in_maps(x1, ei1, x2, ei2, Wg1, bg1, Wf1, bf1, Wg2, bg2, Wf2, bf2,
                  W1, b1, W2, b2, Wo, bo):
    shared = {
        "wg1": np.ascontiguousarray(Wg1.astype(_FP8 if FP8_MM1 else _BF16)),
        "wg2": np.ascontiguousarray(Wg2.astype(_FP8 if FP8_MM1 else _BF16)),
        "wf1": np.ascontiguousarray((Wf1 / float(N)).astype(np.float32)),
        "wf2": np.ascontiguousarray((Wf2 / float(N)).astype(np.float32)),
        "bf1": bf1.reshape(D, 1).astype(np.float32),
        "bf2": bf2.reshape(D, 1).astype(np.float32),
        "bg1": np.ascontiguousarray(bg1.reshape(KT, 128).T.astype(np.float32)),
        "bg2": np.ascontiguousarray(bg2.reshape(KT, 128).T.astype(np.float32)),
        "w1": np.ascontiguousarray(W1.astype(np.float32)),
        "b1": np.ascontiguousarray(b1.reshape(2, 128).T.astype(np.float32)),
        "w2": np.ascontiguousarray(W2.astype(np.float32)),
        "b2": b2.reshape(64, 1).astype(np.float32),
        "wo": Wo.reshape(64, 1).astype(np.float32),
        "bo": bo.reshape(1, 1).astype(np.float32),
    }
    in_maps = []
    for b in range(B):
        m = dict(shared)
        m["xt1"], m["at1"] = _prep_branch(x1[b], ei1[b])
        m["xt2"], m["at2"] = _prep_branch(x2[b], ei2[b])
        in_maps.append(m)
    return in_maps


def kernel(**inputs):
    from concourse.bass_utils import run_bass_kernel_spmd

    nc = _get_nc()
    in_maps = _make_in_maps(**{k: np.asarray(v) for k, v in inputs.items()})
    res = run_bass_kernel_spmd(nc, in_maps, core_ids=list(range(B)))
    out = np.stack([res.results[c]["out"].reshape(1) for c in range(B)], axis=0)
    return out.astype(np.float32)
